# revision 45
# baseline (speedup 1.0000x reference)
"""Trainium2 Bass kernel for nn_Block_78993038508729 (dense transformer
block: rmsnorm -> causal MHA (+degenerate rope) -> rmsnorm -> top-2 MoE
with SwiGLU experts).

Strategy (8 NeuronCores, two launches; host does the O(T*D) elementwise
glue between them -- norms, routing, gathers, residual adds):

  Launch A (attention, bf16): tensor-parallel over heads, 2 heads/core.
    The host precomputes hT = rmsnorm(x)*norm1_w transposed to [D, T]
    (so no device-side rmsnorm, no PE transposes, no sqrt/square
    activation-table ping-pong).  Each core projects its q/k head
    columns into [hd, tok] layout and v directly into [tok, hd] layout,
    runs causal softmax attention with the denominators carried as an
    extra ones-column through the AV matmul, and emits its partial of
    y @ wo in bf16.  Host sums the 8 partials and adds the residual.

  Host: rmsnorm2 + router + exact top-2 + per-expert token gather
    (routing is data-dependent; this is unshard/shard work).

  Launch B (experts, fp8 DoubleRow): expert-parallel, one expert/core.
    Tokens and weights are pre-quantized to fp8e4m3 on the host and
    packed in DoubleRow pair layout [128, 2, .] so every matmul runs at
    2 rows/cycle.  silu on Act, g*u on DVE (fp8 out), down-projection
    also DoubleRow.  Host scatter-adds the weighted expert outputs.

Scheduling: attention is software-pipelined with a 2-pair scores
lookahead, and the neighbouring blocks' projection / output-projection
work is spread between attention pairs as PE fillers so the tensor
engine stays fed while the activation engine works through the exps.
The causal-mask multiplies and the denominator-reciprocal broadcasts
run on the otherwise idle GPSIMD engine.  DMA issue order is arranged
so the first projection's operands land first and output writebacks
never block input streams.

Note on rope: the reference's rope slices freqs[:NH] and broadcasts over
the sequence axis, so the rotation for each head is constant across
positions and identical for q and k.  A fixed orthogonal rotation
applied to both operands of a dot product cancels, so attention scores
-- and therefore the block output -- are unchanged by skipping it.

Numerics (validated against the reference inputs offline): bf16
attention + fp8 MoE gives rel err ~3e-3 vs the 2e-2 gate.  fp8 anywhere
in the attention path perturbs x2 enough to flip top-2 routing picks,
so attention stays bf16.
"""

import sys

if "/opt/trn_rl_repo" not in sys.path:
    sys.path.insert(0, "/opt/trn_rl_repo")

import math

import ml_dtypes
import numpy as np

import concourse.bass as bass
import concourse.mybir as mybir
import concourse.tile as tile
from concourse import bacc
from concourse.bass_utils import run_bass_kernel_spmd

F32 = mybir.dt.float32
BF16 = mybir.dt.bfloat16
F8 = mybir.dt.float8e4
AF = mybir.ActivationFunctionType
PM = mybir.MatmulPerfMode
BF16_NP = ml_dtypes.bfloat16
F8_NP = ml_dtypes.float8_e4m3fn

B, T, D = 1, 2048, 1024
NH, HD = 16, 64
E, K, H = 8, 2, 2048
LAYER_DEPTH = 12
EPS = 1e-8
NCORES = 8
HPC = NH // NCORES          # heads per core = 2
CW = HPC * HD               # per-core head-column width = 128
CAP = 576                   # token capacity per expert core (max load 547)
MOE_SCALE = 1.0 / math.sqrt(LAYER_DEPTH)

_CACHE: dict = {}
MOE_ROUNDS = 0              # launches of the moe kernel in the last call


def _bacc(n_cores):
    return bacc.Bacc("TRN2", target_bir_lowering=False, debug=False,
                     num_devices=n_cores)


# --------------------------------------------------------------------------
# Launch A: attention (head-sharded, bf16).
# Per-core inputs:
#   hT    [128, 8, T] bf16  normed input transposed: hT[p,c,t]=h[t,128c+p]
#   wqkv  [128, 8, 384] bf16  [wq_c | wk_c | wv_c] for this core's heads,
#                             wqkv[p,c,m] = W[128c+p, m]
#   bqk   [128, 2] f32      col 0 bq_c, col 1 bk_c
#   wo    [128, D] bf16     wo rows for this core's head columns
#   trimask [128, 128] bf16 m[k, q] = 1 iff q >= k
#   onesb [1, 64] bf16      ones row (denominator broadcast outer product)
# Output:
#   part  [T, D] bf16       this core's partial of y @ wo (normalized)
# --------------------------------------------------------------------------

def build_attn():
    nc = _bacc(NCORES)
    hT_d = nc.dram_tensor("hT", [128, D // 128, T], BF16, kind="ExternalInput")
    w_d = {w: nc.dram_tensor(w, [128, D // 128, CW], BF16,
                             kind="ExternalInput") for w in ("wq", "wk", "wv")}
    bqk_d = nc.dram_tensor("bqk", [128, 2], F32, kind="ExternalInput")
    wo_d = nc.dram_tensor("wo", [128, D], BF16, kind="ExternalInput")
    trimask_d = nc.dram_tensor("trimask", [128, 128], BF16,
                               kind="ExternalInput")
    onesb_d = nc.dram_tensor("onesb", [1, 128], BF16, kind="ExternalInput")
    part_d = nc.dram_tensor("part", [T, D], BF16, kind="ExternalOutput")

    NC = D // 128            # contraction chunks = 8
    NJ = T // 512            # query blocks = 4

    with tile.TileContext(nc, num_cores=NCORES) as tc:
        with (
            tc.tile_pool(name="const", bufs=1) as const,
            tc.tile_pool(name="big", bufs=1) as bigp,
            tc.tile_pool(name="et", bufs=4) as etp,
            tc.tile_pool(name="dens", bufs=2) as densp,
            tc.tile_pool(name="out", bufs=6) as outp,
            tc.tile_pool(name="ss", bufs=2, space="PSUM") as ps_s,
            tc.tile_pool(name="pa", bufs=2, space="PSUM") as ps_a,
            tc.tile_pool(name="mm", bufs=2, space="PSUM") as ps_m,
        ):
            # DMA issue order matters: the single DMA-engine pool serves
            # transfers in order, and the first q projection needs the q
            # weights + the first hT block before anything else.
            wqkv = {w: const.tile([128, NC, CW], BF16, name=w)
                    for w in ("wq", "wk", "wv")}
            nc.sync.dma_start(out=wqkv["wq"][:], in_=w_d["wq"][:, :, :])
            hT = bigp.tile([128, NC, T], BF16)
            nc.sync.dma_start(out=hT[:, 0:4, 0:512], in_=hT_d[:, 0:4, 0:512])
            nc.sync.dma_start(out=hT[:, 4:8, 0:512], in_=hT_d[:, 4:8, 0:512])
            nc.sync.dma_start(out=wqkv["wk"][:], in_=w_d["wk"][:, :, :])
            nc.sync.dma_start(out=wqkv["wv"][:], in_=w_d["wv"][:, :, :])
            bqk = const.tile([128, 2], F32)
            nc.sync.dma_start(out=bqk[:], in_=bqk_d[:, :])
            trimask = const.tile([128, 128], BF16)
            nc.sync.dma_start(out=trimask[:], in_=trimask_d[:, :])
            onesb = const.tile([1, 128], BF16)
            nc.sync.dma_start(out=onesb[:], in_=onesb_d[:, :])
            for j in range(1, NJ):
                jsl = bass.ts(j, 512)
                nc.sync.dma_start(out=hT[:, :, jsl], in_=hT_d[:, :, jsl])
            wo = const.tile([128, D], BF16)
            nc.sync.dma_start(out=wo[:], in_=wo_d[:, :])

            qT = bigp.tile([128, T], BF16)
            kT = bigp.tile([128, T], BF16)
            yT = bigp.tile([128, T], BF16)
            # v in [tok, hd] layout, grouped [head, 65] with a ones column
            # at local col 64 of each head group (softmax denominators).
            vdir = bigp.tile([128, T // 128, HPC, HD + 1], BF16)
            nc.vector.memset(vdir[:, :, :, HD], 1.0)

            def proj_qk(j, which):
                """q or k projection for token block j (one chunk)."""
                jsl = bass.ts(j, 512)
                dst, wname, brow = ((qT, "wq", 0), (kT, "wk", 1))[which]
                pq = ps_m.tile([128, 512], F32, tag="mm")
                for c in range(NC):
                    nc.tensor.matmul(pq[:], wqkv[wname][:, c, :],
                                     hT[:, c, jsl],
                                     start=(c == 0), stop=(c == NC - 1))
                nc.vector.tensor_scalar_add(dst[:, jsl], pq[:],
                                            bqk[:, brow:brow + 1])

            def proj_v(i):
                """v projection for token tile i, directly in [tok, hd]."""
                isl = bass.ts(i, 128)
                pv = ps_m.tile([128, 512], F32, tag="mm")
                for c in range(NC):
                    nc.tensor.matmul(pv[:, 0:CW], hT[:, c, isl],
                                     wqkv["wv"][:, c, :],
                                     start=(c == 0), stop=(c == NC - 1))
                nc.vector.tensor_copy(
                    vdir[:, i, :, 0:HD],
                    pv[:, 0:CW].rearrange("p (h d) -> p h d", d=HD))

            def qk_chunks(j):
                return [lambda j=j: proj_qk(j, 0), lambda j=j: proj_qk(j, 1)]

            def v_chunks(j):
                return [lambda i=i: proj_v(i) for i in range(4 * j, 4 * j + 4)]

            def outproj_chunk(i, engines=("v", "v")):
                """partial output projection + writeback for token tile i.
                Two [128,512] psum halves on the small-matmul ring so the
                scores ring is never blocked behind output copies."""
                ot = outp.tile([128, 1024], BF16, tag="ot")
                for half in range(2):
                    po = ps_m.tile([128, 512], F32, tag="mm")
                    nc.tensor.matmul(
                        po[:], yT[:, bass.ts(i, 128)],
                        wo[:, 512 * half:512 * (half + 1)],
                        start=True, stop=True)
                    dst = ot[:, 512 * half:512 * (half + 1)]
                    if engines[half] == "v":
                        nc.vector.tensor_copy(dst, po[:])
                    else:
                        nc.scalar.copy(dst, po[:])
                nc.sync.dma_start(out=part_d[bass.ts(i, 128), :], in_=ot[:])

            def outproj_chunks(j):
                return [lambda i=i: outproj_chunk(i)
                        for i in range(4 * j, 4 * j + 4)]

            def attention(j, fillers):
                """causal attention for query block j, both heads.

                Software-pipelined: the scores+exp of pair i+1 are emitted
                before the AV matmuls of pair i, so the PE always has
                score work queued while the Act engine runs exp.  The
                `fillers` (next block's projections, previous block's
                output projection) are spread between pairs to soak up
                the PE idle time while Act works through the exps.
                """
                jsl = bass.ts(j, 512)
                nblk = 4 * j + 4
                # head-interleaved: consecutive items accumulate into
                # different pacc tiles, so their chains overlap.
                items = [(h, ib0) for ib0 in range(0, nblk, 2)
                         for h in range(HPC)]
                paccs = {}
                ets = {}

                def stage_scores(h, ib0):
                    hsl = slice(h * HD, (h + 1) * HD)
                    if ib0 == 0:
                        paccs[h] = ps_a.tile([HD + 1, 512], F32, tag="pacc",
                                             name=f"pacc{h}")
                    pss = ps_s.tile([128, 1024], F32, tag="ss")
                    et = etp.tile([128, 1024], BF16, tag="et")
                    ets[(h, ib0)] = et
                    offs = []
                    for half, ib in enumerate((ib0, ib0 + 1)):
                        off = max(0, (ib - 4 * j) * 128)
                        offs.append(off)
                        nc.tensor.matmul(
                            pss[:, 512 * half + off:512 * (half + 1)],
                            kT[hsl, bass.ts(ib, 128)],
                            qT[hsl, jsl][:, off:512],
                            start=True, stop=True)
                    nc.scalar.activation(
                        out=et[:, offs[0]:1024], in_=pss[:, offs[0]:1024],
                        func=AF.Exp, scale=1.0 / math.sqrt(HD))
                    for half, ib in enumerate((ib0, ib0 + 1)):
                        off = offs[half]
                        if ib >= 4 * j:  # triangular boundary strip (Pool)
                            nc.gpsimd.tensor_mul(
                                et[:, 512 * half + off:512 * half + off + 128],
                                et[:, 512 * half + off:512 * half + off + 128],
                                trimask[:])

                def stage_av(h, ib0):
                    hsl = slice(h * HD, (h + 1) * HD)
                    pacc = paccs[h]
                    et = ets.pop((h, ib0))
                    for half, ib in enumerate((ib0, ib0 + 1)):
                        off = max(0, (ib - 4 * j) * 128)
                        nc.tensor.matmul(
                            pacc[:, off:512], vdir[:, ib, h, :],
                            et[:, 512 * half + off:512 * (half + 1)],
                            start=(ib == 0), stop=(ib == nblk - 1))
                    if ib0 + 2 >= nblk:
                        if j < NJ - 1:
                            # normalize: yT = pacc[0:64] * (1/den); the
                            # reciprocal row is broadcast across partitions
                            # by the (otherwise idle) GPSIMD engine.
                            dr = densp.tile([1, 512], BF16, tag="dr")
                            with nc.allow_low_precision(
                                    reason="bf16 rounding of softmax "
                                           "denominator reciprocals is "
                                           "negligible"):
                                nc.vector.reciprocal(out=dr[:],
                                                     in_=pacc[HD:HD + 1, :])
                            nc.vector.tensor_copy(yT[hsl, jsl],
                                                  pacc[0:HD, :])
                            drb = densp.tile([128, 512], BF16, tag="drb")
                            nc.gpsimd.partition_broadcast(drb[:], dr[0:1, :])
                            nc.vector.tensor_mul(yT[hsl, jsl],
                                                 yT[hsl, jsl], drb[hsl, :])
                        else:
                            norm_pending.append((h, pacc))

                def finish_norms():
                    # final block: both heads' normalizes batched so the
                    # DVE never waits a PE round-trip (recip,recip then
                    # mul,mul), with the PE outer-product broadcast (the
                    # PE is idle here and has lower latency than GPSIMD).
                    drs = []
                    for h, pacc in norm_pending:
                        dr = densp.tile([1, 512], BF16, tag="dr",
                                        name=f"drf{h}")
                        with nc.allow_low_precision(
                                reason="bf16 rounding of softmax "
                                       "denominator reciprocals is "
                                       "negligible"):
                            nc.vector.reciprocal(out=dr[:],
                                                 in_=pacc[HD:HD + 1, :])
                        drs.append(dr)
                    for (h, pacc), dr in zip(norm_pending, drs):
                        hsl = slice(h * HD, (h + 1) * HD)
                        nc.scalar.copy(yT[hsl, jsl], pacc[0:HD, :])
                        pbd = ps_m.tile([128, 512], F32, tag="mm",
                                        name=f"pbdf{h}")
                        nc.tensor.matmul(pbd[:], onesb[:], dr[:],
                                         start=True, stop=True)
                        nc.vector.tensor_mul(yT[hsl, jsl],
                                             yT[hsl, jsl], pbd[hsl, :])

                norm_pending = []
                n = len(items)
                for w in range(min(2, n)):
                    stage_scores(*items[w])
                total = len(fillers)
                done = 0
                for i in range(n):
                    if i + 2 < n:
                        stage_scores(*items[i + 2])
                    target = -(-total * (i + 1) // n)  # ceil fair share
                    while done < target:
                        fillers[done]()
                        done += 1
                    stage_av(*items[i])
                if norm_pending:
                    finish_norms()

            # Block 0's q/k/v run up front.  After that, each block's v
            # projections ride as early fillers of its own attention (the
            # diagonal AV tiles that need them come last), while the next
            # block's q/k and the previous block's output projection fill
            # the rest of the Act-bound stretches.
            for f in qk_chunks(0) + v_chunks(0):
                f()
            for j in range(NJ):
                fill = []
                if j >= 1:
                    fill += v_chunks(j)
                if j + 1 < NJ:
                    fill += qk_chunks(j + 1)
                if j >= 1:
                    fill += outproj_chunks(j - 1)
                attention(j, fill)
            # final block's output projection: both psum rings are free
            # by now, so rotate tiles across them (4-deep pipeline), with
            # the half-copies alternating between both copy engines and
            # per-half DMA writebacks to shorten the tail.
            for i in range(4 * (NJ - 1), 4 * NJ):
                ot = outp.tile([128, 1024], BF16, tag="ot")
                if i % 2 == 0:
                    pow_ = ps_s.tile([128, 1024], F32, tag="ss")
                    pos = [pow_[:, 0:512], pow_[:, 512:1024]]
                else:
                    pos = [ps_m.tile([128, 512], F32, tag="mm",
                                     name=f"poa{i}")[:],
                           ps_m.tile([128, 512], F32, tag="mm",
                                     name=f"pob{i}")[:]]
                for half in range(2):
                    nc.tensor.matmul(
                        pos[half], yT[:, bass.ts(i, 128)],
                        wo[:, 512 * half:512 * (half + 1)],
                        start=True, stop=True)
                    dst = ot[:, 512 * half:512 * (half + 1)]
                    if (i + half) % 2 == 0:
                        nc.vector.tensor_copy(dst, pos[half])
                    else:
                        nc.scalar.copy(dst, pos[half])
                nc.sync.dma_start(out=part_d[bass.ts(i, 128), :], in_=ot[:])
    nc.compile()
    return nc


# --------------------------------------------------------------------------
# Launch B: one expert per core (fp8e4m3 DoubleRow matmuls, f32 psum).
# Per-core inputs:
#   tok8 [128, 8, CAP] fp8   gathered+normed tokens: tok8[p,c,n]=h2[n,128c+p]
#   guw  [16, 128, 8, 256] fp8  per h-tile t: [:,:,0:128]=gate cols,
#                               [:,:,128:256]=up cols, d-major pairs
#   dwn8 [128, 8, 2, D] fp8  down: dwn8[p,hp,i,m]=down[256hp+128i+p, m]
#   wts  [128, 5] f32        routing weight * MOE_SCALE per slot (0 pads)
# Output:
#   eout [CAP, D] bf16       weighted expert output per slot
# --------------------------------------------------------------------------

def build_moe():
    nc = _bacc(NCORES)
    NHT = H // 128           # 16 h tiles
    NTT = (CAP + 127) // 128  # 5 token tiles (last one 64 wide)
    tok8_d = nc.dram_tensor("tok8", [128, D // 128, CAP], F8,
                            kind="ExternalInput")
    guw_d = nc.dram_tensor("guw", [NHT, 128, D // 128, 256], F8,
                           kind="ExternalInput")
    dwn8_d = nc.dram_tensor("dwn8", [128, H // 256, 2, D], F8,
                            kind="ExternalInput")
    wts_d = nc.dram_tensor("wts", [128, NTT], F32, kind="ExternalInput")
    eout_d = nc.dram_tensor("eout", [CAP, D], BF16, kind="ExternalOutput")

    NC2 = D // 256           # 4 DoubleRow d-chunks

    with tile.TileContext(nc, num_cores=NCORES) as tc:
        with (
            tc.tile_pool(name="const", bufs=1) as const,
            tc.tile_pool(name="wstream", bufs=8) as wstream,
            tc.tile_pool(name="gup", bufs=1) as gup,
            tc.tile_pool(name="sg", bufs=2) as sgp,
            tc.tile_pool(name="outp", bufs=3) as outp,
            tc.tile_pool(name="pgu", bufs=3, space="PSUM") as pgu,
            tc.tile_pool(name="po", bufs=2, space="PSUM") as po_p,
        ):
            dwn8 = const.tile([128, H // 256, 2, D], F8)
            guT = gup.tile([128, NHT, CAP], F8)
            tok8 = const.tile([128, D // 128, CAP], F8)
            wts = const.tile([128, NTT], F32)

            # Per-tile gate/up weight DMAs (fine granularity keeps the
            # consumer from waiting on big lumps); tokens right after the
            # first tile, the 2MB down weights last -- they're not needed
            # until the second phase and would stall the gate/up stream.
            gws = []
            for t in range(NHT):
                gw = wstream.tile([128, D // 128, 256], F8, tag="gw",
                                  name=f"gw{t}")
                nc.sync.dma_start(out=gw[:], in_=guw_d[t, :, :, :])
                gws.append(gw)
                if t == 0:
                    nc.sync.dma_start(out=tok8[:, 0:4, :],
                                      in_=tok8_d[:, 0:4, :])
                    nc.sync.dma_start(out=tok8[:, 4:8, :],
                                      in_=tok8_d[:, 4:8, :])
                    nc.sync.dma_start(out=wts[:], in_=wts_d[:, :])
            nc.sync.dma_start(out=dwn8[:], in_=dwn8_d[:, :, :, :])

            for t in range(NHT):
                gw = gws[t]
                # g/u psum: [0:512]=g, [512:1024]=u for the first 512
                # tokens (3-deep ring); the 64-token tail shares the
                # down-projection ring so the main ring stays deep.
                pwA = pgu.tile([128, 1024], F32, tag="guA")
                pwB = po_p.tile([128, 512], F32, tag="o",
                                name=f"pwB{t}")[:, 0:128]
                for gu in range(2):
                    csl = slice(gu * 128, gu * 128 + 128)
                    for c in range(NC2):
                        nc.tensor.matmul(
                            pwA[:, gu * 512:gu * 512 + 512],
                            gw[:, 2 * c:2 * c + 2, csl],
                            tok8[:, 2 * c:2 * c + 2, 0:512],
                            start=(c == 0), stop=(c == NC2 - 1),
                            perf_mode=PM.DoubleRow)
                    for c in range(NC2):
                        nc.tensor.matmul(
                            pwB[:, gu * 64:gu * 64 + 64],
                            gw[:, 2 * c:2 * c + 2, csl],
                            tok8[:, 2 * c:2 * c + 2, 512:CAP],
                            start=(c == 0), stop=(c == NC2 - 1),
                            perf_mode=PM.DoubleRow)
                sg = sgp.tile([128, CAP], BF16, tag="sg")
                nc.scalar.activation(out=sg[:, 0:512], in_=pwA[:, 0:512],
                                     func=AF.Silu)
                nc.scalar.activation(out=sg[:, 512:CAP], in_=pwB[:, 0:64],
                                     func=AF.Silu)
                nc.vector.tensor_mul(guT[:, t, 0:512], sg[:, 0:512],
                                     pwA[:, 512:1024])
                nc.vector.tensor_mul(guT[:, t, 512:CAP], sg[:, 512:CAP],
                                     pwB[:, 64:128])

            for tt in range(NTT):
                ntok = min(128, CAP - tt * 128)
                tsl = slice(tt * 128, tt * 128 + ntok)
                ot = outp.tile([128, D], BF16, tag="ot")
                for half in range(2):
                    dsl = slice(half * 512, half * 512 + 512)
                    pso = po_p.tile([128, 512], F32, tag="o",
                                    name=f"pso{tt}_{half}")
                    for hp in range(H // 256):
                        nc.tensor.matmul(
                            pso[0:ntok, :], guT[:, 2 * hp:2 * hp + 2, tsl],
                            dwn8[:, hp, :, dsl],
                            start=(hp == 0), stop=(hp == H // 256 - 1),
                            perf_mode=PM.DoubleRow)
                    nc.vector.tensor_scalar_mul(ot[0:ntok, dsl],
                                                pso[0:ntok, :],
                                                wts[0:ntok, tt:tt + 1])
                    nc.sync.dma_start(out=eout_d[tsl, dsl],
                                      in_=ot[0:ntok, dsl])
    nc.compile()
    return nc


# --------------------------------------------------------------------------
# Host orchestration
# --------------------------------------------------------------------------

def _get(name, builder):
    if name not in _CACHE:
        _CACHE[name] = builder()
    return _CACHE[name]


def _attn_inputs(x2d, wq, bq, wkv, bkv, wo, norm1_w):
    """Build the 8 per-core input maps for launch A."""
    h = x2d.astype(np.float64)
    h = h / np.sqrt((h * h).mean(axis=-1, keepdims=True) + EPS)
    h = (h * norm1_w.astype(np.float64)).astype(np.float32)
    # hT[p, c, t] = h[t, 128c+p]
    hT = np.ascontiguousarray(
        h.T.reshape(D // 128, 128, T).transpose(1, 0, 2).astype(BF16_NP))

    wk = wkv[:, :D]
    wv = wkv[:, D:]
    bk = bkv[:D]

    tk = np.arange(128)[:, None]
    u = np.arange(128)[None, :]
    trimask = (u >= tk).astype(BF16_NP)
    onesb = np.ones((1, 128), BF16_NP)

    ins = []
    for c in range(NCORES):
        cs = slice(c * CW, (c + 1) * CW)
        packed = {n: np.ascontiguousarray(
            w[:, cs].reshape(D // 128, 128, CW).transpose(1, 0, 2)
            .astype(BF16_NP)) for n, w in (("wq", wq), ("wk", wk),
                                           ("wv", wv))}
        bqk_c = np.ascontiguousarray(
            np.stack([bq[cs], bk[cs]], axis=1).astype(np.float32))
        wo_c = np.ascontiguousarray(wo[cs, :].astype(BF16_NP))
        ins.append({
            "hT": hT,
            **packed,
            "bqk": bqk_c,
            "wo": wo_c,
            "trimask": trimask,
            "onesb": onesb,
        })
    return ins


def _route(x2, router_w, norm2_w):
    """Exact reference routing on host: rmsnorm2 + top-2 + softmax."""
    h2 = x2 / np.sqrt(np.mean(x2 * x2, axis=-1, keepdims=True) + EPS)
    h2 = (h2 * norm2_w).astype(np.float32)
    logits = h2.astype(np.float32) @ router_w.astype(np.float32)   # [N, E]
    idx1 = np.argmax(logits, axis=-1)
    l2 = logits.copy()
    l2[np.arange(T), idx1] = -np.inf
    idx2 = np.argmax(l2, axis=-1)
    v1 = logits[np.arange(T), idx1]
    v2 = logits[np.arange(T), idx2]
    # softmax over the two selected logits (v1 >= v2)
    e2 = np.exp((v2 - v1).astype(np.float32))
    p1 = (1.0 / (1.0 + e2)).astype(np.float32)
    p2 = (e2 / (1.0 + e2)).astype(np.float32)
    return h2, idx1, idx2, p1, p2


def kernel(x, freqs_cos, freqs_sin, norm1_w, wq, bq, wkv, bkv, wo, bo,
           norm2_w, router_w, gate_w, up_w, down_w):
    global MOE_ROUNDS
    x = np.asarray(x, np.float32)
    x2d = np.ascontiguousarray(x.reshape(T, D))
    wq = np.asarray(wq, np.float32)
    wkv = np.asarray(wkv, np.float32)
    wo = np.asarray(wo, np.float32)
    bq = np.asarray(bq, np.float32)
    bkv = np.asarray(bkv, np.float32)
    bo = np.asarray(bo, np.float32)
    norm1_w = np.asarray(norm1_w, np.float32)
    norm2_w = np.asarray(norm2_w, np.float32)
    router_w = np.asarray(router_w, np.float32)
    gate_w = np.asarray(gate_w, np.float32)
    up_w = np.asarray(up_w, np.float32)
    down_w = np.asarray(down_w, np.float32)

    # ---- launch A ----
    nc_a = _get("attn", build_attn)
    ins_a = _attn_inputs(x2d, wq, bq, wkv, bkv, wo, norm1_w)
    res_a = run_bass_kernel_spmd(nc_a, ins_a, core_ids=list(range(NCORES)))
    parts = np.stack([res_a.results[c]["part"].astype(np.float64)
                      for c in range(NCORES)])
    # v-bias folds through attention as +bv (softmax weights sum to 1),
    # so its wo image is added host-side along with bo.
    bv = bkv[D:].astype(np.float64)
    x2 = (x2d.astype(np.float64) + parts.sum(axis=0)
          + bv @ wo.astype(np.float64) + bo.astype(np.float64)
          ).astype(np.float32)

    # ---- host routing ----
    h2, idx1, idx2, p1, p2 = _route(x2, router_w, norm2_w)

    # per-expert token lists (order: top-1 hits then top-2 hits, stable)
    work = []   # (expert, token_idx array, weight array)
    for e in range(E):
        m1 = idx1 == e
        m2 = idx2 == e
        toks = np.concatenate([np.nonzero(m1)[0], np.nonzero(m2)[0]])
        wgts = np.concatenate([p1[m1], p2[m2]]).astype(np.float32)
        for s in range(0, max(len(toks), 1), CAP):
            work.append((e, toks[s:s + CAP], wgts[s:s + CAP]))

    h28 = h2.astype(F8_NP)
    guwb: dict = {}
    dwnb: dict = {}
    NTT = (CAP + 127) // 128

    # ---- launch B (one round of 8 unless an expert overflows CAP) ----
    nc_b = _get("moe", build_moe)
    moe = np.zeros((T, D), np.float64)
    MOE_ROUNDS = 0
    for r0 in range(0, len(work), NCORES):
        batch = work[r0:r0 + NCORES]
        while len(batch) < NCORES:
            batch.append((0, np.zeros(0, np.int64), np.zeros(0, np.float32)))
        ins_b = []
        for e, toks, wgts in batch:
            tok8 = np.zeros((128, D // 128, CAP), F8_NP)
            tok8t = h28[toks].T.reshape(D // 128, 128, len(toks))
            tok8[:, :, :len(toks)] = tok8t.transpose(1, 0, 2)
            wts = np.zeros((NTT * 128,), np.float32)
            wts[:len(toks)] = wgts * MOE_SCALE
            if e not in guwb:
                gu = np.concatenate([
                    gate_w[e].reshape(D, H // 128, 128),
                    up_w[e].reshape(D, H // 128, 128)], axis=2)  # [D,16,256]
                guwb[e] = np.ascontiguousarray(
                    gu.reshape(D // 128, 128, H // 128, 256)
                    .transpose(2, 1, 0, 3).astype(F8_NP))
                dwnb[e] = np.ascontiguousarray(
                    down_w[e].reshape(H // 256, 2, 128, D)
                    .transpose(2, 0, 1, 3).astype(F8_NP))
            ins_b.append({
                "tok8": tok8,
                "guw": guwb[e],
                "dwn8": dwnb[e],
                "wts": np.ascontiguousarray(
                    wts.reshape(NTT, 128).T.astype(np.float32)),
            })
        res_b = run_bass_kernel_spmd(nc_b, ins_b, core_ids=list(range(NCORES)))
        MOE_ROUNDS += 1
        for (e, toks, wgts), rc in zip(batch, res_b.results):
            if len(toks):
                moe[toks] += rc["eout"][:len(toks)].astype(np.float64)

    out = (x2.astype(np.float64) + moe).astype(np.float32)
    return out.reshape(B, T, D)


# revision 49
# speedup vs baseline: 1.0353x; 1.0353x over previous
"""Trainium2 Bass kernel for nn_Block_78993038508729 (dense transformer
block: rmsnorm -> causal MHA (+degenerate rope) -> rmsnorm -> top-2 MoE
with SwiGLU experts).

Strategy (8 NeuronCores, two launches; host does the O(T*D) elementwise
glue between them -- norms, routing, gathers, residual adds):

  Launch A (attention, bf16): tensor-parallel over heads, 2 heads/core.
    The host precomputes hT = rmsnorm(x)*norm1_w transposed to [D, T]
    (so no device-side rmsnorm, no PE transposes, no sqrt/square
    activation-table ping-pong).  Each core projects its q/k head
    columns into [hd, tok] layout and v directly into [tok, hd] layout,
    runs causal softmax attention with the denominators carried as an
    extra ones-column through the AV matmul, and emits its partial of
    y @ wo in bf16.  Host sums the 8 partials and adds the residual.

  Host: rmsnorm2 + router + exact top-2 + per-expert token gather
    (routing is data-dependent; this is unshard/shard work).

  Launch B (experts, fp8 DoubleRow): expert-parallel, one expert/core.
    Tokens and weights are pre-quantized to fp8e4m3 on the host and
    packed in DoubleRow pair layout [128, 2, .] so every matmul runs at
    2 rows/cycle.  silu on Act, g*u on DVE (fp8 out), down-projection
    also DoubleRow.  Host scatter-adds the weighted expert outputs.

Scheduling: attention is software-pipelined with a 2-pair scores
lookahead, and the neighbouring blocks' projection / output-projection
work is spread between attention pairs as PE fillers so the tensor
engine stays fed while the activation engine works through the exps.
The causal-mask multiplies and the denominator-reciprocal broadcasts
run on the otherwise idle GPSIMD engine.  DMA issue order is arranged
so the first projection's operands land first and output writebacks
never block input streams.

Note on rope: the reference's rope slices freqs[:NH] and broadcasts over
the sequence axis, so the rotation for each head is constant across
positions and identical for q and k.  A fixed orthogonal rotation
applied to both operands of a dot product cancels, so attention scores
-- and therefore the block output -- are unchanged by skipping it.

Numerics (validated against the reference inputs offline): bf16
attention + fp8 MoE gives rel err ~3e-3 vs the 2e-2 gate.  fp8 anywhere
in the attention path perturbs x2 enough to flip top-2 routing picks,
so attention stays bf16.
"""

import sys

if "/opt/trn_rl_repo" not in sys.path:
    sys.path.insert(0, "/opt/trn_rl_repo")

import math

import ml_dtypes
import numpy as np

import concourse.bass as bass
import concourse.mybir as mybir
import concourse.tile as tile
from concourse import bacc
from concourse.bass_utils import run_bass_kernel_spmd

F32 = mybir.dt.float32
BF16 = mybir.dt.bfloat16
F8 = mybir.dt.float8e4
AF = mybir.ActivationFunctionType
PM = mybir.MatmulPerfMode
BF16_NP = ml_dtypes.bfloat16
F8_NP = ml_dtypes.float8_e4m3fn

B, T, D = 1, 2048, 1024
NH, HD = 16, 64
E, K, H = 8, 2, 2048
LAYER_DEPTH = 12
EPS = 1e-8
NCORES = 8
HPC = NH // NCORES          # heads per core = 2
CW = HPC * HD               # per-core head-column width = 128
CAP = 576                   # token capacity per expert core (max load 547)
MOE_SCALE = 1.0 / math.sqrt(LAYER_DEPTH)

_CACHE: dict = {}
MOE_ROUNDS = 0              # launches of the moe kernel in the last call


def _bacc(n_cores):
    return bacc.Bacc("TRN2", target_bir_lowering=False, debug=False,
                     num_devices=n_cores)


# --------------------------------------------------------------------------
# Launch A: attention (head-sharded, bf16).
# Per-core inputs:
#   hT    [128, 8, T] bf16  normed input transposed: hT[p,c,t]=h[t,128c+p]
#   wqkv  [128, 8, 384] bf16  [wq_c | wk_c | wv_c] for this core's heads,
#                             wqkv[p,c,m] = W[128c+p, m]
#   bqk   [128, 2] f32      col 0 bq_c, col 1 bk_c
#   wo    [128, D] bf16     wo rows for this core's head columns
#   trimask [128, 128] bf16 m[k, q] = 1 iff q >= k
#   onesb [1, 64] bf16      ones row (denominator broadcast outer product)
# Output:
#   part  [T, D] bf16       this core's partial of y @ wo (normalized)
# --------------------------------------------------------------------------

def build_attn():
    nc = _bacc(NCORES)
    hT_d = nc.dram_tensor("hT", [128, D // 128, T], BF16, kind="ExternalInput")
    w_d = {w: nc.dram_tensor(w, [128, D // 128, CW], BF16,
                             kind="ExternalInput") for w in ("wq", "wk", "wv")}
    bqk_d = nc.dram_tensor("bqk", [128, 2], F32, kind="ExternalInput")
    wo_d = nc.dram_tensor("wo", [128, D], BF16, kind="ExternalInput")
    trimask_d = nc.dram_tensor("trimask", [128, 128], BF16,
                               kind="ExternalInput")
    onesb_d = nc.dram_tensor("onesb", [1, 128], BF16, kind="ExternalInput")
    part_d = nc.dram_tensor("part", [T, D], BF16, kind="ExternalOutput")

    NC = D // 128            # contraction chunks = 8
    NJ = T // 512            # query blocks = 4

    with tile.TileContext(nc, num_cores=NCORES) as tc:
        with (
            tc.tile_pool(name="const", bufs=1) as const,
            tc.tile_pool(name="big", bufs=1) as bigp,
            tc.tile_pool(name="et", bufs=4) as etp,
            tc.tile_pool(name="dens", bufs=2) as densp,
            tc.tile_pool(name="out", bufs=6) as outp,
            tc.tile_pool(name="ss", bufs=2, space="PSUM") as ps_s,
            tc.tile_pool(name="pa", bufs=2, space="PSUM") as ps_a,
            tc.tile_pool(name="mm", bufs=2, space="PSUM") as ps_m,
        ):
            # DMA issue order matters: the single DMA-engine pool serves
            # transfers in order, and the first q projection needs the q
            # weights + the first hT block before anything else.
            wqkv = {w: const.tile([128, NC, CW], BF16, name=w)
                    for w in ("wq", "wk", "wv")}
            nc.sync.dma_start(out=wqkv["wq"][:], in_=w_d["wq"][:, :, :])
            hT = bigp.tile([128, NC, T], BF16)
            nc.sync.dma_start(out=hT[:, 0:4, 0:512], in_=hT_d[:, 0:4, 0:512])
            nc.sync.dma_start(out=hT[:, 4:8, 0:512], in_=hT_d[:, 4:8, 0:512])
            nc.sync.dma_start(out=wqkv["wk"][:], in_=w_d["wk"][:, :, :])
            nc.sync.dma_start(out=wqkv["wv"][:], in_=w_d["wv"][:, :, :])
            bqk = const.tile([128, 2], F32)
            nc.sync.dma_start(out=bqk[:], in_=bqk_d[:, :])
            trimask = const.tile([128, 128], BF16)
            nc.sync.dma_start(out=trimask[:], in_=trimask_d[:, :])
            onesb = const.tile([1, 128], BF16)
            nc.sync.dma_start(out=onesb[:], in_=onesb_d[:, :])
            for j in range(1, NJ):
                jsl = bass.ts(j, 512)
                nc.sync.dma_start(out=hT[:, :, jsl], in_=hT_d[:, :, jsl])
            wo = const.tile([128, D], BF16)
            nc.sync.dma_start(out=wo[:], in_=wo_d[:, :])

            # Warm the PE during the DMA lead-in: the cost of a matmul
            # drops 2-3.7x once the engine has been continuously busy for
            # ~3us, so a train of throwaway matmuls on a zeroed scratch
            # tile brings the first real projections up to full speed.
            warm = bigp.tile([128, 512], BF16)
            nc.vector.memset(warm[:], 0.0)
            pwarm = ps_m.tile([128, 512], F32, tag="mm", name="pwarm")
            for _ in range(9):
                nc.tensor.matmul(pwarm[:], warm[:, 0:128], warm[:],
                                 start=True, stop=True)

            qT = bigp.tile([128, T], BF16)
            kT = bigp.tile([128, T], BF16)
            yT = bigp.tile([128, T], BF16)
            # v in [tok, hd] layout, grouped [head, 65] with a ones column
            # at local col 64 of each head group (softmax denominators).
            vdir = bigp.tile([128, T // 128, HPC, HD + 1], BF16)
            nc.vector.memset(vdir[:, :, :, HD], 1.0)

            def proj_qk(j, which):
                """q or k projection for token block j (one chunk)."""
                jsl = bass.ts(j, 512)
                dst, wname, brow = ((qT, "wq", 0), (kT, "wk", 1))[which]
                pq = ps_m.tile([128, 512], F32, tag="mm")
                for c in range(NC):
                    nc.tensor.matmul(pq[:], wqkv[wname][:, c, :],
                                     hT[:, c, jsl],
                                     start=(c == 0), stop=(c == NC - 1))
                nc.vector.tensor_scalar_add(dst[:, jsl], pq[:],
                                            bqk[:, brow:brow + 1])

            def proj_v(i):
                """v projection for token tile i, directly in [tok, hd]."""
                isl = bass.ts(i, 128)
                pv = ps_m.tile([128, 512], F32, tag="mm")
                for c in range(NC):
                    nc.tensor.matmul(pv[:, 0:CW], hT[:, c, isl],
                                     wqkv["wv"][:, c, :],
                                     start=(c == 0), stop=(c == NC - 1))
                nc.vector.tensor_copy(
                    vdir[:, i, :, 0:HD],
                    pv[:, 0:CW].rearrange("p (h d) -> p h d", d=HD))

            def qk_chunks(j):
                return [lambda j=j: proj_qk(j, 0), lambda j=j: proj_qk(j, 1)]

            def v_chunks(j):
                return [lambda i=i: proj_v(i) for i in range(4 * j, 4 * j + 4)]

            def outproj_chunk(i, engines=("v", "v")):
                """partial output projection + writeback for token tile i.
                Two [128,512] psum halves on the small-matmul ring so the
                scores ring is never blocked behind output copies."""
                ot = outp.tile([128, 1024], BF16, tag="ot")
                for half in range(2):
                    po = ps_m.tile([128, 512], F32, tag="mm")
                    nc.tensor.matmul(
                        po[:], yT[:, bass.ts(i, 128)],
                        wo[:, 512 * half:512 * (half + 1)],
                        start=True, stop=True)
                    dst = ot[:, 512 * half:512 * (half + 1)]
                    if engines[half] == "v":
                        nc.vector.tensor_copy(dst, po[:])
                    else:
                        nc.scalar.copy(dst, po[:])
                nc.sync.dma_start(out=part_d[bass.ts(i, 128), :], in_=ot[:])

            def outproj_chunks(j):
                return [lambda i=i: outproj_chunk(i)
                        for i in range(4 * j, 4 * j + 4)]

            paccs = {}
            ets = {}
            norm_pending = []

            def stage_scores(j, h, ib0):
                jsl = bass.ts(j, 512)
                hsl = slice(h * HD, (h + 1) * HD)
                pss = ps_s.tile([128, 1024], F32, tag="ss")
                et = etp.tile([128, 1024], BF16, tag="et")
                ets[(j, h, ib0)] = et
                offs = []
                for half, ib in enumerate((ib0, ib0 + 1)):
                    off = max(0, (ib - 4 * j) * 128)
                    offs.append(off)
                    nc.tensor.matmul(
                        pss[:, 512 * half + off:512 * (half + 1)],
                        kT[hsl, bass.ts(ib, 128)],
                        qT[hsl, jsl][:, off:512],
                        start=True, stop=True)
                nc.scalar.activation(
                    out=et[:, offs[0]:1024], in_=pss[:, offs[0]:1024],
                    func=AF.Exp, scale=1.0 / math.sqrt(HD))
                for half, ib in enumerate((ib0, ib0 + 1)):
                    off = offs[half]
                    if ib >= 4 * j:  # triangular boundary strip (Pool)
                        nc.gpsimd.tensor_mul(
                            et[:, 512 * half + off:512 * half + off + 128],
                            et[:, 512 * half + off:512 * half + off + 128],
                            trimask[:])

            def stage_av(j, h, ib0):
                jsl = bass.ts(j, 512)
                nblk = 4 * j + 4
                hsl = slice(h * HD, (h + 1) * HD)
                if ib0 == 0:
                    paccs[(j, h)] = ps_a.tile([HD + 1, 512], F32,
                                              tag="pacc", name=f"pacc{j}_{h}")
                pacc = paccs[(j, h)]
                et = ets.pop((j, h, ib0))
                for half, ib in enumerate((ib0, ib0 + 1)):
                    off = max(0, (ib - 4 * j) * 128)
                    nc.tensor.matmul(
                        pacc[:, off:512], vdir[:, ib, h, :],
                        et[:, 512 * half + off:512 * (half + 1)],
                        start=(ib == 0), stop=(ib == nblk - 1))
                if ib0 + 2 >= nblk:
                    if j < NJ - 1:
                        # normalize: yT = pacc[0:64] * (1/den); the
                        # reciprocal row is broadcast across partitions
                        # by the (otherwise idle) GPSIMD engine.
                        dr = densp.tile([1, 512], BF16, tag="dr")
                        with nc.allow_low_precision(
                                reason="bf16 rounding of softmax "
                                       "denominator reciprocals is "
                                       "negligible"):
                            nc.vector.reciprocal(out=dr[:],
                                                 in_=pacc[HD:HD + 1, :])
                        nc.vector.tensor_copy(yT[hsl, jsl], pacc[0:HD, :])
                        drb = densp.tile([128, 512], BF16, tag="drb")
                        nc.gpsimd.partition_broadcast(drb[:], dr[0:1, :])
                        nc.vector.tensor_mul(yT[hsl, jsl],
                                             yT[hsl, jsl], drb[hsl, :])
                    else:
                        # final block: reciprocal + raw copy start right
                        # away; the broadcast-multiply is batched in
                        # finish_norms so the DVE never waits on a PE
                        # round-trip between the two heads.
                        dr = densp.tile([1, 512], BF16, tag="dr",
                                        name=f"drf{h}")
                        with nc.allow_low_precision(
                                reason="bf16 rounding of softmax "
                                       "denominator reciprocals is "
                                       "negligible"):
                            nc.vector.reciprocal(out=dr[:],
                                                 in_=pacc[HD:HD + 1, :])
                        nc.scalar.copy(yT[hsl, bass.ts(j, 512)],
                                       pacc[0:HD, :])
                        norm_pending.append((h, pacc, dr))

            def finish_norms():
                # final block: PE outer-product broadcast of the pre-
                # computed reciprocals (the PE is idle here and has lower
                # latency than GPSIMD), then the two multiplies.
                jsl = bass.ts(NJ - 1, 512)
                for (h, pacc, dr) in norm_pending:
                    hsl = slice(h * HD, (h + 1) * HD)
                    pbd = ps_m.tile([128, 512], F32, tag="mm",
                                    name=f"pbdf{h}")
                    nc.tensor.matmul(pbd[:], onesb[:], dr[:],
                                     start=True, stop=True)
                    nc.vector.tensor_mul(yT[hsl, jsl],
                                         yT[hsl, jsl], pbd[hsl, :])

            # Flat cross-block pipeline.  Block 0's q/k/v run up front;
            # after that each block's v projections, the next block's q/k
            # and the previous block's output projection are spread
            # between that block's attention pairs as PE fillers, and the
            # scores+exp lookahead flows across block boundaries so the
            # Act engine never drains at a block edge.  The next block's
            # q/k fillers are force-completed before its first scores are
            # emitted (the in-order PE queue would otherwise deadlock).
            for f in qk_chunks(0) + v_chunks(0):
                f()
            all_items = []
            fillers = {}
            pos_in_block = {}
            for j in range(NJ):
                blk = [(j, h, ib0) for ib0 in range(0, 4 * j + 4, 2)
                       for h in range(HPC)]
                for p, it in enumerate(blk):
                    pos_in_block[it] = (p, len(blk))
                all_items += blk
                fl = []
                qk_needed = 0
                if j >= 1:
                    fl += v_chunks(j)
                if j + 1 < NJ:
                    fl += qk_chunks(j + 1)
                    qk_needed = len(fl)
                if j >= 1:
                    fl += outproj_chunks(j - 1)
                fillers[j] = [fl, 0, qk_needed]

            def pop_fillers(j, upto):
                fl, done, qk_needed = fillers[j]
                while done < upto and done < len(fl):
                    fl[done]()
                    done += 1
                fillers[j][1] = done

            nitems = len(all_items)
            for w in range(min(2, nitems)):
                stage_scores(*all_items[w])
            for idx in range(nitems):
                j = all_items[idx][0]
                if idx + 2 < nitems:
                    jn = all_items[idx + 2][0]
                    if jn != j:
                        # entering block jn's scores: its q/k must be out
                        pop_fillers(j, fillers[j][2])
                    stage_scores(*all_items[idx + 2])
                p, n = pos_in_block[all_items[idx]]
                pop_fillers(j, -(-len(fillers[j][0]) * (p + 1) // n))
                stage_av(*all_items[idx])
            finish_norms()
            # final block's output projection: both psum rings are free
            # by now, so rotate tiles across them (4-deep pipeline), with
            # the half-copies alternating between both copy engines and
            # per-half DMA writebacks to shorten the tail.
            for i in range(4 * (NJ - 1), 4 * NJ):
                ot = outp.tile([128, 1024], BF16, tag="ot")
                if i % 2 == 0:
                    pow_ = ps_s.tile([128, 1024], F32, tag="ss")
                    pos = [pow_[:, 0:512], pow_[:, 512:1024]]
                else:
                    pos = [ps_m.tile([128, 512], F32, tag="mm",
                                     name=f"poa{i}")[:],
                           ps_m.tile([128, 512], F32, tag="mm",
                                     name=f"pob{i}")[:]]
                for half in range(2):
                    nc.tensor.matmul(
                        pos[half], yT[:, bass.ts(i, 128)],
                        wo[:, 512 * half:512 * (half + 1)],
                        start=True, stop=True)
                    dst = ot[:, 512 * half:512 * (half + 1)]
                    if (i + half) % 2 == 0:
                        nc.vector.tensor_copy(dst, pos[half])
                    else:
                        nc.scalar.copy(dst, pos[half])
                nc.sync.dma_start(out=part_d[bass.ts(i, 128), :], in_=ot[:])
    nc.compile()
    return nc


# --------------------------------------------------------------------------
# Launch B: one expert per core (fp8e4m3 DoubleRow matmuls, f32 psum).
# Per-core inputs:
#   tok8 [128, 8, CAP] fp8   gathered+normed tokens: tok8[p,c,n]=h2[n,128c+p]
#   guw  [16, 128, 8, 256] fp8  per h-tile t: [:,:,0:128]=gate cols,
#                               [:,:,128:256]=up cols, d-major pairs
#   dwn8 [128, 8, 2, D] fp8  down: dwn8[p,hp,i,m]=down[256hp+128i+p, m]
#   wts  [128, 5] f32        routing weight * MOE_SCALE per slot (0 pads)
# Output:
#   eout [CAP, D] bf16       weighted expert output per slot
# --------------------------------------------------------------------------

def build_moe():
    nc = _bacc(NCORES)
    NHT = H // 128           # 16 h tiles
    NTT = (CAP + 127) // 128  # 5 token tiles (last one 64 wide)
    tok8_d = nc.dram_tensor("tok8", [128, D // 128, CAP], F8,
                            kind="ExternalInput")
    guw_d = nc.dram_tensor("guw", [NHT, 128, D // 128, 256], F8,
                           kind="ExternalInput")
    dwn8_d = nc.dram_tensor("dwn8", [128, H // 256, 2, D], F8,
                            kind="ExternalInput")
    wts_d = nc.dram_tensor("wts", [128, NTT], F32, kind="ExternalInput")
    eout_d = nc.dram_tensor("eout", [CAP, D], BF16, kind="ExternalOutput")

    NC2 = D // 256           # 4 DoubleRow d-chunks

    with tile.TileContext(nc, num_cores=NCORES) as tc:
        with (
            tc.tile_pool(name="const", bufs=1) as const,
            tc.tile_pool(name="wstream", bufs=8) as wstream,
            tc.tile_pool(name="gup", bufs=1) as gup,
            tc.tile_pool(name="sg", bufs=2) as sgp,
            tc.tile_pool(name="outp", bufs=3) as outp,
            tc.tile_pool(name="pgu", bufs=3, space="PSUM") as pgu,
            tc.tile_pool(name="po", bufs=2, space="PSUM") as po_p,
        ):
            dwn8 = const.tile([128, H // 256, 2, D], F8)
            guT = gup.tile([128, NHT, CAP], F8)
            tok8 = const.tile([128, D // 128, CAP], F8)
            wts = const.tile([128, NTT], F32)

            # Warm the PE during the DMA lead-in (see build_attn).
            warm = sgp.tile([128, 512], BF16, name="warm", bufs=1)
            nc.vector.memset(warm[:], 0.0)
            pwarm = po_p.tile([128, 512], F32, tag="o", name="pwarm")
            for _ in range(8):
                nc.tensor.matmul(pwarm[:], warm[:, 0:128], warm[:],
                                 start=True, stop=True)

            # Per-tile gate/up weight DMAs (fine granularity keeps the
            # consumer from waiting on big lumps); tokens right after the
            # first tile, the 2MB down weights last -- they're not needed
            # until the second phase and would stall the gate/up stream.
            gws = []
            for t in range(NHT):
                gw = wstream.tile([128, D // 128, 256], F8, tag="gw",
                                  name=f"gw{t}")
                nc.sync.dma_start(out=gw[:], in_=guw_d[t, :, :, :])
                gws.append(gw)
                if t == 0:
                    nc.sync.dma_start(out=tok8[:, 0:4, :],
                                      in_=tok8_d[:, 0:4, :])
                    nc.sync.dma_start(out=tok8[:, 4:8, :],
                                      in_=tok8_d[:, 4:8, :])
                    nc.sync.dma_start(out=wts[:], in_=wts_d[:, :])
            nc.sync.dma_start(out=dwn8[:], in_=dwn8_d[:, :, :, :])

            for t in range(NHT):
                gw = gws[t]
                # g/u psum: [0:512]=g, [512:1024]=u for the first 512
                # tokens (3-deep ring); the 64-token tail shares the
                # down-projection ring so the main ring stays deep.
                pwA = pgu.tile([128, 1024], F32, tag="guA")
                pwB = po_p.tile([128, 512], F32, tag="o",
                                name=f"pwB{t}")[:, 0:128]
                for gu in range(2):
                    csl = slice(gu * 128, gu * 128 + 128)
                    for c in range(NC2):
                        nc.tensor.matmul(
                            pwA[:, gu * 512:gu * 512 + 512],
                            gw[:, 2 * c:2 * c + 2, csl],
                            tok8[:, 2 * c:2 * c + 2, 0:512],
                            start=(c == 0), stop=(c == NC2 - 1),
                            perf_mode=PM.DoubleRow)
                    for c in range(NC2):
                        nc.tensor.matmul(
                            pwB[:, gu * 64:gu * 64 + 64],
                            gw[:, 2 * c:2 * c + 2, csl],
                            tok8[:, 2 * c:2 * c + 2, 512:CAP],
                            start=(c == 0), stop=(c == NC2 - 1),
                            perf_mode=PM.DoubleRow)
                sg = sgp.tile([128, CAP], BF16, tag="sg")
                nc.scalar.activation(out=sg[:, 0:512], in_=pwA[:, 0:512],
                                     func=AF.Silu)
                nc.scalar.activation(out=sg[:, 512:CAP], in_=pwB[:, 0:64],
                                     func=AF.Silu)
                nc.vector.tensor_mul(guT[:, t, 0:512], sg[:, 0:512],
                                     pwA[:, 512:1024])
                nc.vector.tensor_mul(guT[:, t, 512:CAP], sg[:, 512:CAP],
                                     pwB[:, 64:128])

            for tt in range(NTT):
                ntok = min(128, CAP - tt * 128)
                tsl = slice(tt * 128, tt * 128 + ntok)
                ot = outp.tile([128, D], BF16, tag="ot")
                for half in range(2):
                    dsl = slice(half * 512, half * 512 + 512)
                    pso = po_p.tile([128, 512], F32, tag="o",
                                    name=f"pso{tt}_{half}")
                    for hp in range(H // 256):
                        nc.tensor.matmul(
                            pso[0:ntok, :], guT[:, 2 * hp:2 * hp + 2, tsl],
                            dwn8[:, hp, :, dsl],
                            start=(hp == 0), stop=(hp == H // 256 - 1),
                            perf_mode=PM.DoubleRow)
                    nc.vector.tensor_scalar_mul(ot[0:ntok, dsl],
                                                pso[0:ntok, :],
                                                wts[0:ntok, tt:tt + 1])
                    nc.sync.dma_start(out=eout_d[tsl, dsl],
                                      in_=ot[0:ntok, dsl])
    nc.compile()
    return nc


# --------------------------------------------------------------------------
# Host orchestration
# --------------------------------------------------------------------------

def _get(name, builder):
    if name not in _CACHE:
        _CACHE[name] = builder()
    return _CACHE[name]


def _attn_inputs(x2d, wq, bq, wkv, bkv, wo, norm1_w):
    """Build the 8 per-core input maps for launch A."""
    h = x2d.astype(np.float64)
    h = h / np.sqrt((h * h).mean(axis=-1, keepdims=True) + EPS)
    h = (h * norm1_w.astype(np.float64)).astype(np.float32)
    # hT[p, c, t] = h[t, 128c+p]
    hT = np.ascontiguousarray(
        h.T.reshape(D // 128, 128, T).transpose(1, 0, 2).astype(BF16_NP))

    wk = wkv[:, :D]
    wv = wkv[:, D:]
    bk = bkv[:D]

    tk = np.arange(128)[:, None]
    u = np.arange(128)[None, :]
    trimask = (u >= tk).astype(BF16_NP)
    onesb = np.ones((1, 128), BF16_NP)

    ins = []
    for c in range(NCORES):
        cs = slice(c * CW, (c + 1) * CW)
        packed = {n: np.ascontiguousarray(
            w[:, cs].reshape(D // 128, 128, CW).transpose(1, 0, 2)
            .astype(BF16_NP)) for n, w in (("wq", wq), ("wk", wk),
                                           ("wv", wv))}
        bqk_c = np.ascontiguousarray(
            np.stack([bq[cs], bk[cs]], axis=1).astype(np.float32))
        wo_c = np.ascontiguousarray(wo[cs, :].astype(BF16_NP))
        ins.append({
            "hT": hT,
            **packed,
            "bqk": bqk_c,
            "wo": wo_c,
            "trimask": trimask,
            "onesb": onesb,
        })
    return ins


def _route(x2, router_w, norm2_w):
    """Exact reference routing on host: rmsnorm2 + top-2 + softmax."""
    h2 = x2 / np.sqrt(np.mean(x2 * x2, axis=-1, keepdims=True) + EPS)
    h2 = (h2 * norm2_w).astype(np.float32)
    logits = h2.astype(np.float32) @ router_w.astype(np.float32)   # [N, E]
    idx1 = np.argmax(logits, axis=-1)
    l2 = logits.copy()
    l2[np.arange(T), idx1] = -np.inf
    idx2 = np.argmax(l2, axis=-1)
    v1 = logits[np.arange(T), idx1]
    v2 = logits[np.arange(T), idx2]
    # softmax over the two selected logits (v1 >= v2)
    e2 = np.exp((v2 - v1).astype(np.float32))
    p1 = (1.0 / (1.0 + e2)).astype(np.float32)
    p2 = (e2 / (1.0 + e2)).astype(np.float32)
    return h2, idx1, idx2, p1, p2


def kernel(x, freqs_cos, freqs_sin, norm1_w, wq, bq, wkv, bkv, wo, bo,
           norm2_w, router_w, gate_w, up_w, down_w):
    global MOE_ROUNDS
    x = np.asarray(x, np.float32)
    x2d = np.ascontiguousarray(x.reshape(T, D))
    wq = np.asarray(wq, np.float32)
    wkv = np.asarray(wkv, np.float32)
    wo = np.asarray(wo, np.float32)
    bq = np.asarray(bq, np.float32)
    bkv = np.asarray(bkv, np.float32)
    bo = np.asarray(bo, np.float32)
    norm1_w = np.asarray(norm1_w, np.float32)
    norm2_w = np.asarray(norm2_w, np.float32)
    router_w = np.asarray(router_w, np.float32)
    gate_w = np.asarray(gate_w, np.float32)
    up_w = np.asarray(up_w, np.float32)
    down_w = np.asarray(down_w, np.float32)

    # ---- launch A ----
    nc_a = _get("attn", build_attn)
    ins_a = _attn_inputs(x2d, wq, bq, wkv, bkv, wo, norm1_w)
    res_a = run_bass_kernel_spmd(nc_a, ins_a, core_ids=list(range(NCORES)))
    parts = np.stack([res_a.results[c]["part"].astype(np.float64)
                      for c in range(NCORES)])
    # v-bias folds through attention as +bv (softmax weights sum to 1),
    # so its wo image is added host-side along with bo.
    bv = bkv[D:].astype(np.float64)
    x2 = (x2d.astype(np.float64) + parts.sum(axis=0)
          + bv @ wo.astype(np.float64) + bo.astype(np.float64)
          ).astype(np.float32)

    # ---- host routing ----
    h2, idx1, idx2, p1, p2 = _route(x2, router_w, norm2_w)

    # per-expert token lists (order: top-1 hits then top-2 hits, stable)
    work = []   # (expert, token_idx array, weight array)
    for e in range(E):
        m1 = idx1 == e
        m2 = idx2 == e
        toks = np.concatenate([np.nonzero(m1)[0], np.nonzero(m2)[0]])
        wgts = np.concatenate([p1[m1], p2[m2]]).astype(np.float32)
        for s in range(0, max(len(toks), 1), CAP):
            work.append((e, toks[s:s + CAP], wgts[s:s + CAP]))

    h28 = h2.astype(F8_NP)
    guwb: dict = {}
    dwnb: dict = {}
    NTT = (CAP + 127) // 128

    # ---- launch B (one round of 8 unless an expert overflows CAP) ----
    nc_b = _get("moe", build_moe)
    moe = np.zeros((T, D), np.float64)
    MOE_ROUNDS = 0
    for r0 in range(0, len(work), NCORES):
        batch = work[r0:r0 + NCORES]
        while len(batch) < NCORES:
            batch.append((0, np.zeros(0, np.int64), np.zeros(0, np.float32)))
        ins_b = []
        for e, toks, wgts in batch:
            tok8 = np.zeros((128, D // 128, CAP), F8_NP)
            tok8t = h28[toks].T.reshape(D // 128, 128, len(toks))
            tok8[:, :, :len(toks)] = tok8t.transpose(1, 0, 2)
            wts = np.zeros((NTT * 128,), np.float32)
            wts[:len(toks)] = wgts * MOE_SCALE
            if e not in guwb:
                gu = np.concatenate([
                    gate_w[e].reshape(D, H // 128, 128),
                    up_w[e].reshape(D, H // 128, 128)], axis=2)  # [D,16,256]
                guwb[e] = np.ascontiguousarray(
                    gu.reshape(D // 128, 128, H // 128, 256)
                    .transpose(2, 1, 0, 3).astype(F8_NP))
                dwnb[e] = np.ascontiguousarray(
                    down_w[e].reshape(H // 256, 2, 128, D)
                    .transpose(2, 0, 1, 3).astype(F8_NP))
            ins_b.append({
                "tok8": tok8,
                "guw": guwb[e],
                "dwn8": dwnb[e],
                "wts": np.ascontiguousarray(
                    wts.reshape(NTT, 128).T.astype(np.float32)),
            })
        res_b = run_bass_kernel_spmd(nc_b, ins_b, core_ids=list(range(NCORES)))
        MOE_ROUNDS += 1
        for (e, toks, wgts), rc in zip(batch, res_b.results):
            if len(toks):
                moe[toks] += rc["eout"][:len(toks)].astype(np.float64)

    out = (x2.astype(np.float64) + moe).astype(np.float32)
    return out.reshape(B, T, D)


# revision 57
# speedup vs baseline: 1.0367x; 1.0013x over previous
"""Trainium2 Bass kernel for nn_Block_78993038508729 (dense transformer
block: rmsnorm -> causal MHA (+degenerate rope) -> rmsnorm -> top-2 MoE
with SwiGLU experts).

Strategy (8 NeuronCores, two launches; host does the O(T*D) elementwise
glue between them -- norms, routing, gathers, residual adds):

  Launch A (attention, bf16): tensor-parallel over heads, 2 heads/core.
    The host precomputes hT = rmsnorm(x)*norm1_w transposed to [D, T]
    (so no device-side rmsnorm, no PE transposes, no sqrt/square
    activation-table ping-pong).  Each core projects its q/k head
    columns into [hd, tok] layout and v directly into [tok, hd] layout,
    runs causal softmax attention with the denominators carried as an
    extra ones-column through the AV matmul, and emits its partial of
    y @ wo in bf16.  Host sums the 8 partials and adds the residual.

  Host: rmsnorm2 + router + exact top-2 + per-expert token gather
    (routing is data-dependent; this is unshard/shard work).

  Launch B (experts, fp8 DoubleRow): expert-parallel, one expert/core.
    Tokens and weights are pre-quantized to fp8e4m3 on the host and
    packed in DoubleRow pair layout [128, 2, .] so every matmul runs at
    2 rows/cycle.  silu on Act, g*u on DVE (fp8 out), down-projection
    also DoubleRow.  Host scatter-adds the weighted expert outputs.

Scheduling: attention is software-pipelined with a 2-pair scores
lookahead, and the neighbouring blocks' projection / output-projection
work is spread between attention pairs as PE fillers so the tensor
engine stays fed while the activation engine works through the exps.
The causal-mask multiplies and the denominator-reciprocal broadcasts
run on the otherwise idle GPSIMD engine.  DMA issue order is arranged
so the first projection's operands land first and output writebacks
never block input streams.

Note on rope: the reference's rope slices freqs[:NH] and broadcasts over
the sequence axis, so the rotation for each head is constant across
positions and identical for q and k.  A fixed orthogonal rotation
applied to both operands of a dot product cancels, so attention scores
-- and therefore the block output -- are unchanged by skipping it.

Numerics (validated against the reference inputs offline): bf16
attention + fp8 MoE gives rel err ~3e-3 vs the 2e-2 gate.  fp8 anywhere
in the attention path perturbs x2 enough to flip top-2 routing picks,
so attention stays bf16.
"""

import sys

if "/opt/trn_rl_repo" not in sys.path:
    sys.path.insert(0, "/opt/trn_rl_repo")

import math

import ml_dtypes
import numpy as np

import concourse.bass as bass
import concourse.mybir as mybir
import concourse.tile as tile
from concourse import bacc
from concourse.bass_utils import run_bass_kernel_spmd

F32 = mybir.dt.float32
BF16 = mybir.dt.bfloat16
F8 = mybir.dt.float8e4
AF = mybir.ActivationFunctionType
PM = mybir.MatmulPerfMode
BF16_NP = ml_dtypes.bfloat16
F8_NP = ml_dtypes.float8_e4m3fn

B, T, D = 1, 2048, 1024
NH, HD = 16, 64
E, K, H = 8, 2, 2048
LAYER_DEPTH = 12
EPS = 1e-8
NCORES = 8
HPC = NH // NCORES          # heads per core = 2
CW = HPC * HD               # per-core head-column width = 128
CAP = 576                   # token capacity per expert core (max load 547)
MOE_SCALE = 1.0 / math.sqrt(LAYER_DEPTH)

_CACHE: dict = {}
MOE_ROUNDS = 0              # launches of the moe kernel in the last call


def _bacc(n_cores):
    return bacc.Bacc("TRN2", target_bir_lowering=False, debug=False,
                     num_devices=n_cores)


# --------------------------------------------------------------------------
# Launch A: attention (head-sharded, bf16).
# Per-core inputs:
#   hT    [128, 8, T] bf16  normed input transposed: hT[p,c,t]=h[t,128c+p]
#   wqkv  [128, 8, 384] bf16  [wq_c | wk_c | wv_c] for this core's heads,
#                             wqkv[p,c,m] = W[128c+p, m]
#   bqk   [128, 2] f32      col 0 bq_c, col 1 bk_c
#   wo    [128, D] bf16     wo rows for this core's head columns
#   trimask [128, 128] bf16 m[k, q] = 1 iff q >= k
#   onesb [1, 64] bf16      ones row (denominator broadcast outer product)
# Output:
#   part  [T, D] bf16       this core's partial of y @ wo (normalized)
# --------------------------------------------------------------------------

def build_attn():
    nc = _bacc(NCORES)
    hT_d = nc.dram_tensor("hT", [128, D // 128, T], BF16, kind="ExternalInput")
    w_d = {w: nc.dram_tensor(w, [128, D // 128, CW], BF16,
                             kind="ExternalInput") for w in ("wq", "wk", "wv")}
    bqk_d = nc.dram_tensor("bqk", [128, 2], F32, kind="ExternalInput")
    wo_d = nc.dram_tensor("wo", [128, D], BF16, kind="ExternalInput")
    trimask_d = nc.dram_tensor("trimask", [128, 128], BF16,
                               kind="ExternalInput")
    onesb_d = nc.dram_tensor("onesb", [1, 128], BF16, kind="ExternalInput")
    part_d = nc.dram_tensor("part", [T, D], BF16, kind="ExternalOutput")

    NC = D // 128            # contraction chunks = 8
    NJ = T // 512            # query blocks = 4

    with tile.TileContext(nc, num_cores=NCORES) as tc:
        with (
            tc.tile_pool(name="const", bufs=1) as const,
            tc.tile_pool(name="big", bufs=1) as bigp,
            tc.tile_pool(name="et", bufs=4) as etp,
            tc.tile_pool(name="dens", bufs=2) as densp,
            tc.tile_pool(name="out", bufs=6) as outp,
            tc.tile_pool(name="ss", bufs=2, space="PSUM") as ps_s,
            tc.tile_pool(name="pa", bufs=2, space="PSUM") as ps_a,
            tc.tile_pool(name="mm", bufs=2, space="PSUM") as ps_m,
        ):
            # DMA issue order matters: the single DMA-engine pool serves
            # transfers in order, and the first q projection needs the q
            # weights + the first hT block before anything else.
            wqkv = {w: const.tile([128, NC, CW], BF16, name=w)
                    for w in ("wq", "wk", "wv")}
            nc.sync.dma_start(out=wqkv["wq"][:], in_=w_d["wq"][:, :, :])
            hT = bigp.tile([128, NC, T], BF16)
            nc.sync.dma_start(out=hT[:, 0:4, 0:512], in_=hT_d[:, 0:4, 0:512])
            nc.sync.dma_start(out=hT[:, 4:8, 0:512], in_=hT_d[:, 4:8, 0:512])
            nc.sync.dma_start(out=wqkv["wk"][:], in_=w_d["wk"][:, :, :])
            nc.sync.dma_start(out=wqkv["wv"][:], in_=w_d["wv"][:, :, :])
            bqk = const.tile([128, 2], F32)
            nc.sync.dma_start(out=bqk[:], in_=bqk_d[:, :])
            trimask = const.tile([128, 128], BF16)
            nc.sync.dma_start(out=trimask[:], in_=trimask_d[:, :])
            onesb = const.tile([1, 128], BF16)
            nc.sync.dma_start(out=onesb[:], in_=onesb_d[:, :])
            for j in range(1, NJ):
                jsl = bass.ts(j, 512)
                nc.sync.dma_start(out=hT[:, :, jsl], in_=hT_d[:, :, jsl])
            wo = const.tile([128, D], BF16)
            nc.sync.dma_start(out=wo[:], in_=wo_d[:, :])

            # Warm the PE during the DMA lead-in: the cost of a matmul
            # drops 2-3.7x once the engine has been continuously busy for
            # ~3us, so a train of throwaway matmuls on a zeroed scratch
            # tile brings the first real projections up to full speed.
            warm = bigp.tile([128, 512], BF16)
            nc.vector.memset(warm[:], 0.0)
            pwarm = ps_m.tile([128, 512], F32, tag="mm", name="pwarm")
            for _ in range(9):
                nc.tensor.matmul(pwarm[:], warm[:, 0:128], warm[:],
                                 start=True, stop=True)

            qT = bigp.tile([128, T], BF16)
            kT = bigp.tile([128, T], BF16)
            yT = bigp.tile([128, T], BF16)
            # v in [tok, hd] layout, grouped [head, 65] with a ones column
            # at local col 64 of each head group (softmax denominators).
            vdir = bigp.tile([128, T // 128, HPC, HD + 1], BF16)
            nc.vector.memset(vdir[:, :, :, HD], 1.0)

            def proj_qk(j, which):
                """q or k projection for token block j (one chunk)."""
                jsl = bass.ts(j, 512)
                dst, wname, brow = ((qT, "wq", 0), (kT, "wk", 1))[which]
                pq = ps_m.tile([128, 512], F32, tag="mm")
                for c in range(NC):
                    nc.tensor.matmul(pq[:], wqkv[wname][:, c, :],
                                     hT[:, c, jsl],
                                     start=(c == 0), stop=(c == NC - 1))
                nc.vector.tensor_scalar_add(dst[:, jsl], pq[:],
                                            bqk[:, brow:brow + 1])

            def proj_v(i):
                """v projection for token tile i, directly in [tok, hd]."""
                isl = bass.ts(i, 128)
                pv = ps_m.tile([128, 512], F32, tag="mm")
                for c in range(NC):
                    nc.tensor.matmul(pv[:, 0:CW], hT[:, c, isl],
                                     wqkv["wv"][:, c, :],
                                     start=(c == 0), stop=(c == NC - 1))
                nc.vector.tensor_copy(
                    vdir[:, i, :, 0:HD],
                    pv[:, 0:CW].rearrange("p (h d) -> p h d", d=HD))

            def qk_chunks(j):
                return [lambda j=j: proj_qk(j, 0), lambda j=j: proj_qk(j, 1)]

            def v_chunks(j):
                return [lambda i=i: proj_v(i) for i in range(4 * j, 4 * j + 4)]

            def outproj_chunk(i, engines=("v", "v")):
                """partial output projection + writeback for token tile i.
                Two [128,512] psum halves on the small-matmul ring so the
                scores ring is never blocked behind output copies."""
                ot = outp.tile([128, 1024], BF16, tag="ot")
                for half in range(2):
                    po = ps_m.tile([128, 512], F32, tag="mm")
                    nc.tensor.matmul(
                        po[:], yT[:, bass.ts(i, 128)],
                        wo[:, 512 * half:512 * (half + 1)],
                        start=True, stop=True)
                    dst = ot[:, 512 * half:512 * (half + 1)]
                    if engines[half] == "v":
                        nc.vector.tensor_copy(dst, po[:])
                    else:
                        nc.scalar.copy(dst, po[:])
                nc.sync.dma_start(out=part_d[bass.ts(i, 128), :], in_=ot[:])

            def outproj_chunks(j):
                return [lambda i=i: outproj_chunk(i)
                        for i in range(4 * j, 4 * j + 4)]

            paccs = {}
            ets = {}
            norm_pending = []

            def stage_scores(j, h, ib0):
                jsl = bass.ts(j, 512)
                hsl = slice(h * HD, (h + 1) * HD)
                pss = ps_s.tile([128, 1024], F32, tag="ss")
                et = etp.tile([128, 1024], BF16, tag="et")
                ets[(j, h, ib0)] = et
                offs = []
                for half, ib in enumerate((ib0, ib0 + 1)):
                    off = max(0, (ib - 4 * j) * 128)
                    offs.append(off)
                    nc.tensor.matmul(
                        pss[:, 512 * half + off:512 * (half + 1)],
                        kT[hsl, bass.ts(ib, 128)],
                        qT[hsl, jsl][:, off:512],
                        start=True, stop=True)
                nc.scalar.activation(
                    out=et[:, offs[0]:1024], in_=pss[:, offs[0]:1024],
                    func=AF.Exp, scale=1.0 / math.sqrt(HD))
                for half, ib in enumerate((ib0, ib0 + 1)):
                    off = offs[half]
                    if ib >= 4 * j:  # triangular boundary strip (Pool)
                        nc.gpsimd.tensor_mul(
                            et[:, 512 * half + off:512 * half + off + 128],
                            et[:, 512 * half + off:512 * half + off + 128],
                            trimask[:])

            def stage_av(j, h, ib0):
                jsl = bass.ts(j, 512)
                nblk = 4 * j + 4
                hsl = slice(h * HD, (h + 1) * HD)
                if ib0 == 0:
                    paccs[(j, h)] = ps_a.tile([HD + 1, 512], F32,
                                              tag="pacc", name=f"pacc{j}_{h}")
                pacc = paccs[(j, h)]
                et = ets.pop((j, h, ib0))
                for half, ib in enumerate((ib0, ib0 + 1)):
                    off = max(0, (ib - 4 * j) * 128)
                    nc.tensor.matmul(
                        pacc[:, off:512], vdir[:, ib, h, :],
                        et[:, 512 * half + off:512 * (half + 1)],
                        start=(ib == 0), stop=(ib == nblk - 1))
                if ib0 + 2 >= nblk:
                    if j < NJ - 1:
                        # normalize: yT = pacc[0:64] * (1/den); the
                        # reciprocal row is broadcast across partitions
                        # by the (otherwise idle) GPSIMD engine.
                        dr = densp.tile([1, 512], BF16, tag="dr")
                        with nc.allow_low_precision(
                                reason="bf16 rounding of softmax "
                                       "denominator reciprocals is "
                                       "negligible"):
                            nc.vector.reciprocal(out=dr[:],
                                                 in_=pacc[HD:HD + 1, :])
                        nc.vector.tensor_copy(yT[hsl, jsl], pacc[0:HD, :])
                        drb = densp.tile([128, 512], BF16, tag="drb")
                        nc.gpsimd.partition_broadcast(drb[:], dr[0:1, :])
                        nc.vector.tensor_mul(yT[hsl, jsl],
                                             yT[hsl, jsl], drb[hsl, :])
                    else:
                        # final block: reciprocal + raw copy start right
                        # away; the broadcast-multiply is batched in
                        # finish_norms so the DVE never waits on a PE
                        # round-trip between the two heads.
                        dr = densp.tile([1, 512], BF16, tag="dr",
                                        name=f"drf{h}")
                        with nc.allow_low_precision(
                                reason="bf16 rounding of softmax "
                                       "denominator reciprocals is "
                                       "negligible"):
                            nc.vector.reciprocal(out=dr[:],
                                                 in_=pacc[HD:HD + 1, :])
                        nc.scalar.copy(yT[hsl, bass.ts(j, 512)],
                                       pacc[0:HD, :])
                        norm_pending.append((h, pacc, dr))

            def finish_norms():
                # final block: PE outer-product broadcast of the pre-
                # computed reciprocals (the PE is idle here and has lower
                # latency than GPSIMD), then the two multiplies.
                jsl = bass.ts(NJ - 1, 512)
                for (h, pacc, dr) in norm_pending:
                    hsl = slice(h * HD, (h + 1) * HD)
                    pbd = ps_m.tile([128, 512], F32, tag="mm",
                                    name=f"pbdf{h}")
                    nc.tensor.matmul(pbd[:], onesb[:], dr[:],
                                     start=True, stop=True)
                    nc.vector.tensor_mul(yT[hsl, jsl],
                                         yT[hsl, jsl], pbd[hsl, :])

            # Flat cross-block pipeline.  Block 0's q/k/v run up front;
            # after that each block's v projections, the next block's q/k
            # and the previous block's output projection are spread
            # between that block's attention pairs as PE fillers, and the
            # scores+exp lookahead flows across block boundaries so the
            # Act engine never drains at a block edge.  The next block's
            # q/k fillers are force-completed before its first scores are
            # emitted (the in-order PE queue would otherwise deadlock).
            for f in qk_chunks(0) + v_chunks(0):
                f()
            all_items = []
            fillers = {}
            pos_in_block = {}
            for j in range(NJ):
                blk = [(j, h, ib0) for ib0 in range(0, 4 * j + 4, 2)
                       for h in range(HPC)]
                for p, it in enumerate(blk):
                    pos_in_block[it] = (p, len(blk))
                all_items += blk
                fl = []
                qk_needed = 0
                if j >= 1:
                    fl += v_chunks(j)
                if j + 1 < NJ:
                    fl += qk_chunks(j + 1)
                    qk_needed = len(fl)
                if j >= 1:
                    fl += outproj_chunks(j - 1)
                fillers[j] = [fl, 0, qk_needed]

            def pop_fillers(j, upto):
                fl, done, qk_needed = fillers[j]
                while done < upto and done < len(fl):
                    fl[done]()
                    done += 1
                fillers[j][1] = done

            LOOK = 3
            nitems = len(all_items)
            for w in range(min(LOOK, nitems)):
                stage_scores(*all_items[w])
            for idx in range(nitems):
                j = all_items[idx][0]
                if idx + LOOK < nitems:
                    jn = all_items[idx + LOOK][0]
                    if jn != j:
                        # entering block jn's scores: its q/k must be out
                        pop_fillers(j, fillers[j][2])
                    stage_scores(*all_items[idx + LOOK])
                p, n = pos_in_block[all_items[idx]]
                pop_fillers(j, -(-len(fillers[j][0]) * (p + 1) // n))
                stage_av(*all_items[idx])
            finish_norms()
            # final block's output projection: both psum rings are free
            # by now, so rotate tiles across them (4-deep pipeline), with
            # the half-copies alternating between both copy engines and
            # per-half DMA writebacks to shorten the tail.
            for i in range(4 * (NJ - 1), 4 * NJ):
                ot = outp.tile([128, 1024], BF16, tag="ot")
                if i % 2 == 0:
                    pow_ = ps_s.tile([128, 1024], F32, tag="ss")
                    pos = [pow_[:, 0:512], pow_[:, 512:1024]]
                else:
                    pos = [ps_m.tile([128, 512], F32, tag="mm",
                                     name=f"poa{i}")[:],
                           ps_m.tile([128, 512], F32, tag="mm",
                                     name=f"pob{i}")[:]]
                for half in range(2):
                    nc.tensor.matmul(
                        pos[half], yT[:, bass.ts(i, 128)],
                        wo[:, 512 * half:512 * (half + 1)],
                        start=True, stop=True)
                    dst = ot[:, 512 * half:512 * (half + 1)]
                    if (i + half) % 2 == 0:
                        nc.vector.tensor_copy(dst, pos[half])
                    else:
                        nc.scalar.copy(dst, pos[half])
                nc.sync.dma_start(out=part_d[bass.ts(i, 128), :], in_=ot[:])
    nc.compile()
    return nc


# --------------------------------------------------------------------------
# Launch B: one expert per core (fp8e4m3 DoubleRow matmuls, f32 psum).
# Per-core inputs:
#   tok8 [128, 8, CAP] fp8   gathered+normed tokens: tok8[p,c,n]=h2[n,128c+p]
#   guw  [16, 128, 8, 256] fp8  per h-tile t: [:,:,0:128]=gate cols,
#                               [:,:,128:256]=up cols, d-major pairs
#   dwn8 [128, 8, 2, D] fp8  down: dwn8[p,hp,i,m]=down[256hp+128i+p, m]
#   wts  [128, 5] f32        routing weight * MOE_SCALE per slot (0 pads)
# Output:
#   eout [CAP, D] bf16       weighted expert output per slot
# --------------------------------------------------------------------------

def build_moe():
    nc = _bacc(NCORES)
    NHT = H // 128           # 16 h tiles
    NTT = (CAP + 127) // 128  # 5 token tiles (last one 64 wide)
    tok8_d = nc.dram_tensor("tok8", [128, D // 128, CAP], F8,
                            kind="ExternalInput")
    guw_d = nc.dram_tensor("guw", [NHT, 128, D // 128, 256], F8,
                           kind="ExternalInput")
    dwn8_d = nc.dram_tensor("dwn8", [128, H // 256, 2, D], F8,
                            kind="ExternalInput")
    wts_d = nc.dram_tensor("wts", [128, NTT], F32, kind="ExternalInput")
    eout_d = nc.dram_tensor("eout", [CAP, D], BF16, kind="ExternalOutput")

    NC2 = D // 256           # 4 DoubleRow d-chunks

    with tile.TileContext(nc, num_cores=NCORES) as tc:
        with (
            tc.tile_pool(name="const", bufs=1) as const,
            tc.tile_pool(name="wstream", bufs=8) as wstream,
            tc.tile_pool(name="gup", bufs=1) as gup,
            tc.tile_pool(name="sg", bufs=2) as sgp,
            tc.tile_pool(name="outp", bufs=3) as outp,
            tc.tile_pool(name="pgu", bufs=3, space="PSUM") as pgu,
            tc.tile_pool(name="po", bufs=2, space="PSUM") as po_p,
        ):
            dwn8 = const.tile([128, H // 256, 2, D], F8)
            guT = gup.tile([128, NHT, CAP], F8)
            tok8 = const.tile([128, D // 128, CAP], F8)
            wts = const.tile([128, NTT], F32)

            # Warm the PE during the DMA lead-in (see build_attn).
            warm = sgp.tile([128, 512], BF16, name="warm", bufs=1)
            nc.vector.memset(warm[:], 0.0)
            pwarm = po_p.tile([128, 512], F32, tag="o", name="pwarm")
            for _ in range(8):
                nc.tensor.matmul(pwarm[:], warm[:, 0:128], warm[:],
                                 start=True, stop=True)

            # Per-tile gate/up weight DMAs (fine granularity keeps the
            # consumer from waiting on big lumps); tokens right after the
            # first tile, the 2MB down weights last -- they're not needed
            # until the second phase and would stall the gate/up stream.
            gws = []
            for t in range(NHT):
                gw = wstream.tile([128, D // 128, 256], F8, tag="gw",
                                  name=f"gw{t}")
                nc.sync.dma_start(out=gw[:], in_=guw_d[t, :, :, :])
                gws.append(gw)
                if t == 0:
                    nc.sync.dma_start(out=tok8[:, 0:4, :],
                                      in_=tok8_d[:, 0:4, :])
                    nc.sync.dma_start(out=tok8[:, 4:8, :],
                                      in_=tok8_d[:, 4:8, :])
                    nc.sync.dma_start(out=wts[:], in_=wts_d[:, :])
            nc.sync.dma_start(out=dwn8[:], in_=dwn8_d[:, :, :, :])

            for t in range(NHT):
                gw = gws[t]
                # g/u psum: [0:512]=g, [512:1024]=u for the first 512
                # tokens (3-deep ring); the 64-token tail shares the
                # down-projection ring so the main ring stays deep.
                pwA = pgu.tile([128, 1024], F32, tag="guA")
                pwB = po_p.tile([128, 512], F32, tag="o",
                                name=f"pwB{t}")[:, 0:128]
                for gu in range(2):
                    csl = slice(gu * 128, gu * 128 + 128)
                    for c in range(NC2):
                        nc.tensor.matmul(
                            pwA[:, gu * 512:gu * 512 + 512],
                            gw[:, 2 * c:2 * c + 2, csl],
                            tok8[:, 2 * c:2 * c + 2, 0:512],
                            start=(c == 0), stop=(c == NC2 - 1),
                            perf_mode=PM.DoubleRow)
                    for c in range(NC2):
                        nc.tensor.matmul(
                            pwB[:, gu * 64:gu * 64 + 64],
                            gw[:, 2 * c:2 * c + 2, csl],
                            tok8[:, 2 * c:2 * c + 2, 512:CAP],
                            start=(c == 0), stop=(c == NC2 - 1),
                            perf_mode=PM.DoubleRow)
                sg = sgp.tile([128, CAP], BF16, tag="sg")
                nc.scalar.activation(out=sg[:, 0:512], in_=pwA[:, 0:512],
                                     func=AF.Silu)
                nc.scalar.activation(out=sg[:, 512:CAP], in_=pwB[:, 0:64],
                                     func=AF.Silu)
                nc.vector.tensor_mul(guT[:, t, 0:512], sg[:, 0:512],
                                     pwA[:, 512:1024])
                nc.vector.tensor_mul(guT[:, t, 512:CAP], sg[:, 512:CAP],
                                     pwB[:, 64:128])

            for tt in range(NTT):
                ntok = min(128, CAP - tt * 128)
                tsl = slice(tt * 128, tt * 128 + ntok)
                ot = outp.tile([128, D], BF16, tag="ot")
                for half in range(2):
                    dsl = slice(half * 512, half * 512 + 512)
                    pso = po_p.tile([128, 512], F32, tag="o",
                                    name=f"pso{tt}_{half}")
                    for hp in range(H // 256):
                        nc.tensor.matmul(
                            pso[0:ntok, :], guT[:, 2 * hp:2 * hp + 2, tsl],
                            dwn8[:, hp, :, dsl],
                            start=(hp == 0), stop=(hp == H // 256 - 1),
                            perf_mode=PM.DoubleRow)
                    nc.vector.tensor_scalar_mul(ot[0:ntok, dsl],
                                                pso[0:ntok, :],
                                                wts[0:ntok, tt:tt + 1])
                    nc.sync.dma_start(out=eout_d[tsl, dsl],
                                      in_=ot[0:ntok, dsl])
    nc.compile()
    return nc


# --------------------------------------------------------------------------
# Host orchestration
# --------------------------------------------------------------------------

def _get(name, builder):
    if name not in _CACHE:
        _CACHE[name] = builder()
    return _CACHE[name]


def _attn_inputs(x2d, wq, bq, wkv, bkv, wo, norm1_w):
    """Build the 8 per-core input maps for launch A."""
    h = x2d.astype(np.float64)
    h = h / np.sqrt((h * h).mean(axis=-1, keepdims=True) + EPS)
    h = (h * norm1_w.astype(np.float64)).astype(np.float32)
    # hT[p, c, t] = h[t, 128c+p]
    hT = np.ascontiguousarray(
        h.T.reshape(D // 128, 128, T).transpose(1, 0, 2).astype(BF16_NP))

    wk = wkv[:, :D]
    wv = wkv[:, D:]
    bk = bkv[:D]

    tk = np.arange(128)[:, None]
    u = np.arange(128)[None, :]
    trimask = (u >= tk).astype(BF16_NP)
    onesb = np.ones((1, 128), BF16_NP)

    ins = []
    for c in range(NCORES):
        cs = slice(c * CW, (c + 1) * CW)
        packed = {n: np.ascontiguousarray(
            w[:, cs].reshape(D // 128, 128, CW).transpose(1, 0, 2)
            .astype(BF16_NP)) for n, w in (("wq", wq), ("wk", wk),
                                           ("wv", wv))}
        bqk_c = np.ascontiguousarray(
            np.stack([bq[cs], bk[cs]], axis=1).astype(np.float32))
        wo_c = np.ascontiguousarray(wo[cs, :].astype(BF16_NP))
        ins.append({
            "hT": hT,
            **packed,
            "bqk": bqk_c,
            "wo": wo_c,
            "trimask": trimask,
            "onesb": onesb,
        })
    return ins


def _route(x2, router_w, norm2_w):
    """Exact reference routing on host: rmsnorm2 + top-2 + softmax."""
    h2 = x2 / np.sqrt(np.mean(x2 * x2, axis=-1, keepdims=True) + EPS)
    h2 = (h2 * norm2_w).astype(np.float32)
    logits = h2.astype(np.float32) @ router_w.astype(np.float32)   # [N, E]
    idx1 = np.argmax(logits, axis=-1)
    l2 = logits.copy()
    l2[np.arange(T), idx1] = -np.inf
    idx2 = np.argmax(l2, axis=-1)
    v1 = logits[np.arange(T), idx1]
    v2 = logits[np.arange(T), idx2]
    # softmax over the two selected logits (v1 >= v2)
    e2 = np.exp((v2 - v1).astype(np.float32))
    p1 = (1.0 / (1.0 + e2)).astype(np.float32)
    p2 = (e2 / (1.0 + e2)).astype(np.float32)
    return h2, idx1, idx2, p1, p2


def kernel(x, freqs_cos, freqs_sin, norm1_w, wq, bq, wkv, bkv, wo, bo,
           norm2_w, router_w, gate_w, up_w, down_w):
    global MOE_ROUNDS
    x = np.asarray(x, np.float32)
    x2d = np.ascontiguousarray(x.reshape(T, D))
    wq = np.asarray(wq, np.float32)
    wkv = np.asarray(wkv, np.float32)
    wo = np.asarray(wo, np.float32)
    bq = np.asarray(bq, np.float32)
    bkv = np.asarray(bkv, np.float32)
    bo = np.asarray(bo, np.float32)
    norm1_w = np.asarray(norm1_w, np.float32)
    norm2_w = np.asarray(norm2_w, np.float32)
    router_w = np.asarray(router_w, np.float32)
    gate_w = np.asarray(gate_w, np.float32)
    up_w = np.asarray(up_w, np.float32)
    down_w = np.asarray(down_w, np.float32)

    # ---- launch A ----
    nc_a = _get("attn", build_attn)
    ins_a = _attn_inputs(x2d, wq, bq, wkv, bkv, wo, norm1_w)
    res_a = run_bass_kernel_spmd(nc_a, ins_a, core_ids=list(range(NCORES)))
    parts = np.stack([res_a.results[c]["part"].astype(np.float64)
                      for c in range(NCORES)])
    # v-bias folds through attention as +bv (softmax weights sum to 1),
    # so its wo image is added host-side along with bo.
    bv = bkv[D:].astype(np.float64)
    x2 = (x2d.astype(np.float64) + parts.sum(axis=0)
          + bv @ wo.astype(np.float64) + bo.astype(np.float64)
          ).astype(np.float32)

    # ---- host routing ----
    h2, idx1, idx2, p1, p2 = _route(x2, router_w, norm2_w)

    # per-expert token lists (order: top-1 hits then top-2 hits, stable)
    work = []   # (expert, token_idx array, weight array)
    for e in range(E):
        m1 = idx1 == e
        m2 = idx2 == e
        toks = np.concatenate([np.nonzero(m1)[0], np.nonzero(m2)[0]])
        wgts = np.concatenate([p1[m1], p2[m2]]).astype(np.float32)
        for s in range(0, max(len(toks), 1), CAP):
            work.append((e, toks[s:s + CAP], wgts[s:s + CAP]))

    h28 = h2.astype(F8_NP)
    guwb: dict = {}
    dwnb: dict = {}
    NTT = (CAP + 127) // 128

    # ---- launch B (one round of 8 unless an expert overflows CAP) ----
    nc_b = _get("moe", build_moe)
    moe = np.zeros((T, D), np.float64)
    MOE_ROUNDS = 0
    for r0 in range(0, len(work), NCORES):
        batch = work[r0:r0 + NCORES]
        while len(batch) < NCORES:
            batch.append((0, np.zeros(0, np.int64), np.zeros(0, np.float32)))
        ins_b = []
        for e, toks, wgts in batch:
            tok8 = np.zeros((128, D // 128, CAP), F8_NP)
            tok8t = h28[toks].T.reshape(D // 128, 128, len(toks))
            tok8[:, :, :len(toks)] = tok8t.transpose(1, 0, 2)
            wts = np.zeros((NTT * 128,), np.float32)
            wts[:len(toks)] = wgts * MOE_SCALE
            if e not in guwb:
                gu = np.concatenate([
                    gate_w[e].reshape(D, H // 128, 128),
                    up_w[e].reshape(D, H // 128, 128)], axis=2)  # [D,16,256]
                guwb[e] = np.ascontiguousarray(
                    gu.reshape(D // 128, 128, H // 128, 256)
                    .transpose(2, 1, 0, 3).astype(F8_NP))
                dwnb[e] = np.ascontiguousarray(
                    down_w[e].reshape(H // 256, 2, 128, D)
                    .transpose(2, 0, 1, 3).astype(F8_NP))
            ins_b.append({
                "tok8": tok8,
                "guw": guwb[e],
                "dwn8": dwnb[e],
                "wts": np.ascontiguousarray(
                    wts.reshape(NTT, 128).T.astype(np.float32)),
            })
        res_b = run_bass_kernel_spmd(nc_b, ins_b, core_ids=list(range(NCORES)))
        MOE_ROUNDS += 1
        for (e, toks, wgts), rc in zip(batch, res_b.results):
            if len(toks):
                moe[toks] += rc["eout"][:len(toks)].astype(np.float64)

    out = (x2.astype(np.float64) + moe).astype(np.float32)
    return out.reshape(B, T, D)


# revision 77
# speedup vs baseline: 1.0555x; 1.0181x over previous
"""Trainium2 Bass kernel for nn_Block_78993038508729 (dense transformer
block: rmsnorm -> causal MHA (+degenerate rope) -> rmsnorm -> top-2 MoE
with SwiGLU experts).

Strategy (8 NeuronCores, two launches; host does the O(T*D) elementwise
glue between them -- norms, routing, gathers, residual adds):

  Launch A (attention, bf16): tensor-parallel over heads, 2 heads/core.
    The host precomputes hT = rmsnorm(x)*norm1_w transposed to [D, T]
    (so no device-side rmsnorm, no PE transposes, no sqrt/square
    activation-table ping-pong).  Each core projects its q/k head
    columns into [hd, tok] layout and v directly into [tok, hd] layout,
    runs causal softmax attention with the denominators carried as an
    extra ones-column through the AV matmul, and emits its partial of
    y @ wo in bf16.  Host sums the 8 partials and adds the residual.

  Host: rmsnorm2 + router + exact top-2 + per-expert token gather
    (routing is data-dependent; this is unshard/shard work).

  Launch B (experts, fp8 DoubleRow): expert-parallel, one expert/core.
    Tokens and weights are pre-quantized to fp8e4m3 on the host and
    packed in DoubleRow pair layout [128, 2, .] so every matmul runs at
    2 rows/cycle.  silu on Act, g*u on DVE (fp8 out), down-projection
    also DoubleRow.  Host scatter-adds the weighted expert outputs.

Scheduling: attention is software-pipelined with a 2-pair scores
lookahead, and the neighbouring blocks' projection / output-projection
work is spread between attention pairs as PE fillers so the tensor
engine stays fed while the activation engine works through the exps.
The causal-mask multiplies and the denominator-reciprocal broadcasts
run on the otherwise idle GPSIMD engine.  DMA issue order is arranged
so the first projection's operands land first and output writebacks
never block input streams.

Note on rope: the reference's rope slices freqs[:NH] and broadcasts over
the sequence axis, so the rotation for each head is constant across
positions and identical for q and k.  A fixed orthogonal rotation
applied to both operands of a dot product cancels, so attention scores
-- and therefore the block output -- are unchanged by skipping it.

Numerics (validated against the reference inputs offline): bf16
attention + fp8 MoE gives rel err ~3e-3 vs the 2e-2 gate.  fp8 anywhere
in the attention path perturbs x2 enough to flip top-2 routing picks,
so attention stays bf16.
"""

import sys

if "/opt/trn_rl_repo" not in sys.path:
    sys.path.insert(0, "/opt/trn_rl_repo")

import math

import ml_dtypes
import numpy as np

import concourse.bass as bass
import concourse.mybir as mybir
import concourse.tile as tile
from concourse import bacc
from concourse.bass_utils import run_bass_kernel_spmd

F32 = mybir.dt.float32
BF16 = mybir.dt.bfloat16
F8 = mybir.dt.float8e4
AF = mybir.ActivationFunctionType
PM = mybir.MatmulPerfMode
BF16_NP = ml_dtypes.bfloat16
F8_NP = ml_dtypes.float8_e4m3fn

B, T, D = 1, 2048, 1024
NH, HD = 16, 64
E, K, H = 8, 2, 2048
LAYER_DEPTH = 12
EPS = 1e-8
NCORES = 8
HPC = NH // NCORES          # heads per core = 2
CW = HPC * HD               # per-core head-column width = 128
CAP = 576                   # token capacity per expert core (max load 547)
MOE_SCALE = 1.0 / math.sqrt(LAYER_DEPTH)

_CACHE: dict = {}
MOE_ROUNDS = 0              # launches of the moe kernel in the last call


def _bacc(n_cores):
    return bacc.Bacc("TRN2", target_bir_lowering=False, debug=False,
                     num_devices=n_cores)


# --------------------------------------------------------------------------
# Launch A: attention (head-sharded, bf16).
# Per-core inputs:
#   hT    [128, 8, T] bf16  normed input transposed: hT[p,c,t]=h[t,128c+p]
#   wqkv  [128, 8, 384] bf16  [wq_c | wk_c | wv_c] for this core's heads,
#                             wqkv[p,c,m] = W[128c+p, m]
#   bqk   [128, 2] f32      col 0 bq_c, col 1 bk_c
#   wo    [128, D] bf16     wo rows for this core's head columns
#   trimask [128, 128] bf16 m[k, q] = 1 iff q >= k
#   onesb [1, 64] bf16      ones row (denominator broadcast outer product)
# Output:
#   part  [T, D] bf16       this core's partial of y @ wo (normalized)
# --------------------------------------------------------------------------

def build_attn():
    nc = _bacc(NCORES)
    hT_d = nc.dram_tensor("hT", [128, D // 128, T], BF16, kind="ExternalInput")
    w_d = {w: nc.dram_tensor(w, [128, D // 128, CW], BF16,
                             kind="ExternalInput") for w in ("wq", "wk", "wv")}
    bqk_d = nc.dram_tensor("bqk", [128, 2], F32, kind="ExternalInput")
    wo_d = nc.dram_tensor("wo", [128, D], BF16, kind="ExternalInput")
    trimask_d = nc.dram_tensor("trimask", [128, 128], BF16,
                               kind="ExternalInput")
    onesb_d = nc.dram_tensor("onesb", [1, 128], BF16, kind="ExternalInput")
    part_d = nc.dram_tensor("part", [T, D], BF16, kind="ExternalOutput")

    NC = D // 128            # contraction chunks = 8
    NJ = T // 512            # query blocks = 4

    with tile.TileContext(nc, num_cores=NCORES) as tc:
        with (
            tc.tile_pool(name="const", bufs=1) as const,
            tc.tile_pool(name="big", bufs=1) as bigp,
            tc.tile_pool(name="et", bufs=6) as etp,
            tc.tile_pool(name="dens", bufs=4) as densp,
            tc.tile_pool(name="out", bufs=6) as outp,
            tc.tile_pool(name="ss", bufs=2, space="PSUM") as ps_s,
            tc.tile_pool(name="pa", bufs=2, space="PSUM") as ps_a,
            tc.tile_pool(name="mm", bufs=2, space="PSUM") as ps_m,
        ):
            # DMA issue order matters: the single DMA-engine pool serves
            # transfers in order, and the first q projection needs the q
            # weights + the first hT block before anything else.
            wqkv = {w: const.tile([128, NC, CW], BF16, name=w)
                    for w in ("wq", "wk", "wv")}
            nc.sync.dma_start(out=wqkv["wq"][:], in_=w_d["wq"][:, :, :])
            hT = bigp.tile([128, NC, T], BF16)
            nc.sync.dma_start(out=hT[:, 0:4, 0:512], in_=hT_d[:, 0:4, 0:512])
            nc.sync.dma_start(out=hT[:, 4:8, 0:512], in_=hT_d[:, 4:8, 0:512])
            nc.sync.dma_start(out=wqkv["wk"][:], in_=w_d["wk"][:, :, :])
            nc.sync.dma_start(out=wqkv["wv"][:], in_=w_d["wv"][:, :, :])
            bqk = const.tile([128, 2], F32)
            nc.sync.dma_start(out=bqk[:], in_=bqk_d[:, :])
            trimask = const.tile([128, 128], BF16)
            nc.sync.dma_start(out=trimask[:], in_=trimask_d[:, :])
            onesb = const.tile([1, 128], BF16)
            nc.sync.dma_start(out=onesb[:], in_=onesb_d[:, :])
            for j in range(1, NJ):
                jsl = bass.ts(j, 512)
                nc.sync.dma_start(out=hT[:, :, jsl], in_=hT_d[:, :, jsl])
            wo = const.tile([128, D], BF16)
            nc.sync.dma_start(out=wo[:], in_=wo_d[:, :])

            # Warm the PE during the DMA lead-in: the cost of a matmul
            # drops 2-3.7x once the engine has been continuously busy for
            # ~3us, so a train of throwaway matmuls on a zeroed scratch
            # tile brings the first real projections up to full speed.
            warm = bigp.tile([128, 512], BF16)
            nc.vector.memset(warm[:], 0.0)
            pwarm = ps_m.tile([128, 512], F32, tag="mm", name="pwarm")
            for _ in range(9):
                nc.tensor.matmul(pwarm[:], warm[:, 0:128], warm[:],
                                 start=True, stop=True)

            qT = bigp.tile([128, T], BF16)
            kT = bigp.tile([128, T], BF16)
            yT = bigp.tile([128, T], BF16)
            # v in [tok, hd] layout, grouped [head, 65] with a ones column
            # at local col 64 of each head group (softmax denominators).
            vdir = bigp.tile([128, T // 128, HPC, HD + 1], BF16)
            nc.vector.memset(vdir[:, :, :, HD], 1.0)

            def proj_qk(j, which):
                """q or k projection for token block j (one chunk)."""
                jsl = bass.ts(j, 512)
                dst, wname, brow = ((qT, "wq", 0), (kT, "wk", 1))[which]
                pq = ps_m.tile([128, 512], F32, tag="mm")
                for c in range(NC):
                    nc.tensor.matmul(pq[:], wqkv[wname][:, c, :],
                                     hT[:, c, jsl],
                                     start=(c == 0), stop=(c == NC - 1))
                nc.vector.tensor_scalar_add(dst[:, jsl], pq[:],
                                            bqk[:, brow:brow + 1])

            def proj_v(i):
                """v projection for token tile i, directly in [tok, hd]."""
                isl = bass.ts(i, 128)
                pv = ps_m.tile([128, 512], F32, tag="mm")
                for c in range(NC):
                    nc.tensor.matmul(pv[:, 0:CW], hT[:, c, isl],
                                     wqkv["wv"][:, c, :],
                                     start=(c == 0), stop=(c == NC - 1))
                nc.vector.tensor_copy(
                    vdir[:, i, :, 0:HD],
                    pv[:, 0:CW].rearrange("p (h d) -> p h d", d=HD))

            def qk_chunks(j):
                return [lambda j=j: proj_qk(j, 0), lambda j=j: proj_qk(j, 1)]

            def v_chunks(j):
                return [lambda i=i: proj_v(i) for i in range(4 * j, 4 * j + 4)]

            def outproj_chunk(i, engines=("v", "v")):
                """partial output projection + writeback for token tile i.
                Two [128,512] psum halves on the small-matmul ring so the
                scores ring is never blocked behind output copies."""
                ot = outp.tile([128, 1024], BF16, tag="ot")
                for half in range(2):
                    po = ps_m.tile([128, 512], F32, tag="mm")
                    nc.tensor.matmul(
                        po[:], yT[:, bass.ts(i, 128)],
                        wo[:, 512 * half:512 * (half + 1)],
                        start=True, stop=True)
                    dst = ot[:, 512 * half:512 * (half + 1)]
                    if engines[half] == "v":
                        nc.vector.tensor_copy(dst, po[:])
                    else:
                        nc.scalar.copy(dst, po[:])
                nc.sync.dma_start(out=part_d[bass.ts(i, 128), :], in_=ot[:])

            def outproj_chunks(j):
                return [lambda i=i: outproj_chunk(i)
                        for i in range(4 * j, 4 * j + 4)]

            paccs = {}
            ets = {}
            norm_pending = []

            def stage_scores(j, h, ib0):
                jsl = bass.ts(j, 512)
                hsl = slice(h * HD, (h + 1) * HD)
                pss = ps_s.tile([128, 1024], F32, tag="ss")
                et = etp.tile([128, 1024], BF16, tag="et")
                ets[(j, h, ib0)] = et
                offs = []
                for half, ib in enumerate((ib0, ib0 + 1)):
                    off = max(0, (ib - 4 * j) * 128)
                    offs.append(off)
                    nc.tensor.matmul(
                        pss[:, 512 * half + off:512 * (half + 1)],
                        kT[hsl, bass.ts(ib, 128)],
                        qT[hsl, jsl][:, off:512],
                        start=True, stop=True)
                nc.scalar.activation(
                    out=et[:, offs[0]:1024], in_=pss[:, offs[0]:1024],
                    func=AF.Exp, scale=1.0 / math.sqrt(HD))
                for half, ib in enumerate((ib0, ib0 + 1)):
                    off = offs[half]
                    if ib >= 4 * j:  # triangular boundary strip (Pool)
                        nc.gpsimd.tensor_mul(
                            et[:, 512 * half + off:512 * half + off + 128],
                            et[:, 512 * half + off:512 * half + off + 128],
                            trimask[:])

            def stage_av(j, h, ib0):
                jsl = bass.ts(j, 512)
                nblk = 4 * j + 4
                hsl = slice(h * HD, (h + 1) * HD)
                if ib0 == 0:
                    paccs[(j, h)] = ps_a.tile([HD + 1, 512], F32,
                                              tag="pacc", name=f"pacc{j}_{h}")
                pacc = paccs[(j, h)]
                et = ets.pop((j, h, ib0))
                for half, ib in enumerate((ib0, ib0 + 1)):
                    off = max(0, (ib - 4 * j) * 128)
                    nc.tensor.matmul(
                        pacc[:, off:512], vdir[:, ib, h, :],
                        et[:, 512 * half + off:512 * (half + 1)],
                        start=(ib == 0), stop=(ib == nblk - 1))
                if ib0 + 2 >= nblk:
                    if j < NJ - 1:
                        # normalize: yT = pacc[0:64] * (1/den); the
                        # reciprocal row is broadcast across partitions
                        # by the (otherwise idle) GPSIMD engine.
                        dr = densp.tile([1, 512], BF16, tag="dr")
                        with nc.allow_low_precision(
                                reason="bf16 rounding of softmax "
                                       "denominator reciprocals is "
                                       "negligible"):
                            nc.vector.reciprocal(out=dr[:],
                                                 in_=pacc[HD:HD + 1, :])
                        nc.vector.tensor_copy(yT[hsl, jsl], pacc[0:HD, :])
                        drb = densp.tile([128, 512], BF16, tag="drb")
                        nc.gpsimd.partition_broadcast(drb[:], dr[0:1, :])
                        nc.vector.tensor_mul(yT[hsl, jsl],
                                             yT[hsl, jsl], drb[hsl, :])
                    else:
                        # final block: reciprocal + raw copy start right
                        # away; the broadcast-multiply is batched in
                        # finish_norms so the DVE never waits on a PE
                        # round-trip between the two heads.
                        dr = densp.tile([1, 512], BF16, tag="dr",
                                        name=f"drf{h}")
                        with nc.allow_low_precision(
                                reason="bf16 rounding of softmax "
                                       "denominator reciprocals is "
                                       "negligible"):
                            nc.vector.reciprocal(out=dr[:],
                                                 in_=pacc[HD:HD + 1, :])
                        nc.scalar.copy(yT[hsl, bass.ts(j, 512)],
                                       pacc[0:HD, :])
                        norm_pending.append((h, pacc, dr))

            def finish_norms():
                # final block: PE outer-product broadcast of the pre-
                # computed reciprocals (the PE is idle here and has lower
                # latency than GPSIMD), then the two multiplies.
                jsl = bass.ts(NJ - 1, 512)
                for (h, pacc, dr) in norm_pending:
                    hsl = slice(h * HD, (h + 1) * HD)
                    pbd = ps_m.tile([128, 512], F32, tag="mm",
                                    name=f"pbdf{h}")
                    nc.tensor.matmul(pbd[:], onesb[:], dr[:],
                                     start=True, stop=True)
                    nc.vector.tensor_mul(yT[hsl, jsl],
                                         yT[hsl, jsl], pbd[hsl, :])

            # Flat cross-block pipeline.  Block 0's q/k/v run up front;
            # after that each block's v projections, the next block's q/k
            # and the previous block's output projection are spread
            # between that block's attention pairs as PE fillers, and the
            # scores+exp lookahead flows across block boundaries so the
            # Act engine never drains at a block edge.  The next block's
            # q/k fillers are force-completed before its first scores are
            # emitted (the in-order PE queue would otherwise deadlock).
            for f in qk_chunks(0) + v_chunks(0):
                f()
            all_items = []
            fillers = {}
            pos_in_block = {}
            for j in range(NJ):
                blk = [(j, h, ib0) for ib0 in range(0, 4 * j + 4, 2)
                       for h in range(HPC)]
                for p, it in enumerate(blk):
                    pos_in_block[it] = (p, len(blk))
                all_items += blk
                fl = []
                qk_needed = 0
                if j >= 1:
                    fl += v_chunks(j)
                if j + 1 < NJ:
                    fl += qk_chunks(j + 1)
                    qk_needed = len(fl)
                if j >= 1:
                    fl += outproj_chunks(j - 1)
                fillers[j] = [fl, 0, qk_needed]

            def pop_fillers(j, upto):
                fl, done, qk_needed = fillers[j]
                while done < upto and done < len(fl):
                    fl[done]()
                    done += 1
                fillers[j][1] = done

            LOOK = 4
            nitems = len(all_items)
            for w in range(min(LOOK, nitems)):
                stage_scores(*all_items[w])
            for idx in range(nitems):
                j = all_items[idx][0]
                if idx + LOOK < nitems:
                    jn = all_items[idx + LOOK][0]
                    if jn != j:
                        # entering block jn's scores: its q/k must be out
                        pop_fillers(j, fillers[j][2])
                    stage_scores(*all_items[idx + LOOK])
                p, n = pos_in_block[all_items[idx]]
                pop_fillers(j, -(-len(fillers[j][0]) * (p + 1) // n))
                stage_av(*all_items[idx])
            finish_norms()
            # final block's output projection: both psum rings are free
            # by now, so rotate tiles across them (4-deep pipeline), with
            # the half-copies alternating between both copy engines and
            # per-half DMA writebacks to shorten the tail.
            for i in range(4 * (NJ - 1), 4 * NJ):
                ot = outp.tile([128, 1024], BF16, tag="ot")
                if i % 2 == 0:
                    pow_ = ps_s.tile([128, 1024], F32, tag="ss")
                    pos = [pow_[:, 0:512], pow_[:, 512:1024]]
                else:
                    pos = [ps_m.tile([128, 512], F32, tag="mm",
                                     name=f"poa{i}")[:],
                           ps_m.tile([128, 512], F32, tag="mm",
                                     name=f"pob{i}")[:]]
                for half in range(2):
                    nc.tensor.matmul(
                        pos[half], yT[:, bass.ts(i, 128)],
                        wo[:, 512 * half:512 * (half + 1)],
                        start=True, stop=True)
                    dst = ot[:, 512 * half:512 * (half + 1)]
                    if (i + half) % 2 == 0:
                        nc.vector.tensor_copy(dst, pos[half])
                    else:
                        nc.scalar.copy(dst, pos[half])
                nc.sync.dma_start(out=part_d[bass.ts(i, 128), :], in_=ot[:])
    nc.compile()
    return nc


# --------------------------------------------------------------------------
# Launch B: one expert per core (fp8e4m3 DoubleRow matmuls, f32 psum).
# Per-core inputs:
#   tok8 [128, 8, CAP] fp8   gathered+normed tokens: tok8[p,c,n]=h2[n,128c+p]
#   guw  [16, 128, 8, 256] fp8  per h-tile t: [:,:,0:128]=gate cols,
#                               [:,:,128:256]=up cols, d-major pairs
#   dwn8 [128, 8, 2, D] fp8  down: dwn8[p,hp,i,m]=down[256hp+128i+p, m]
#   wts  [128, 5] f32        routing weight * MOE_SCALE per slot (0 pads)
# Output:
#   eout [CAP, D] bf16       weighted expert output per slot
# --------------------------------------------------------------------------

def build_moe():
    nc = _bacc(NCORES)
    NHT = H // 128           # 16 h tiles
    NTT = (CAP + 127) // 128  # 5 token tiles (last one 64 wide)
    tok8_d = nc.dram_tensor("tok8", [128, D // 128, CAP], F8,
                            kind="ExternalInput")
    guw_d = nc.dram_tensor("guw", [NHT, 128, D // 128, 256], F8,
                           kind="ExternalInput")
    dwn8_d = nc.dram_tensor("dwn8", [128, H // 256, 2, D], F8,
                            kind="ExternalInput")
    wts_d = nc.dram_tensor("wts", [128, NTT], F32, kind="ExternalInput")
    eout_d = nc.dram_tensor("eout", [CAP, D], BF16, kind="ExternalOutput")

    NC2 = D // 256           # 4 DoubleRow d-chunks

    with tile.TileContext(nc, num_cores=NCORES) as tc:
        with (
            tc.tile_pool(name="const", bufs=1) as const,
            tc.tile_pool(name="wstream", bufs=8) as wstream,
            tc.tile_pool(name="gup", bufs=1) as gup,
            tc.tile_pool(name="sg", bufs=3) as sgp,
            tc.tile_pool(name="outp", bufs=3) as outp,
            tc.tile_pool(name="pgu", bufs=3, space="PSUM") as pgu,
            tc.tile_pool(name="po", bufs=2, space="PSUM") as po_p,
        ):
            dwn8 = const.tile([128, H // 256, 2, D], F8)
            guT = gup.tile([128, NHT, CAP], F8)
            tok8 = const.tile([128, D // 128, CAP], F8)
            wts = const.tile([128, NTT], F32)

            # Warm the PE during the DMA lead-in (see build_attn).
            warm = sgp.tile([128, 512], BF16, name="warm", bufs=1)
            nc.vector.memset(warm[:], 0.0)
            pwarm = po_p.tile([128, 512], F32, tag="o", name="pwarm")
            for _ in range(7):
                nc.tensor.matmul(pwarm[:], warm[:, 0:128], warm[:],
                                 start=True, stop=True)

            # Per-tile gate/up weight DMAs (fine granularity keeps the
            # consumer from waiting on big lumps); tokens right after the
            # first tile, the 2MB down weights last -- they're not needed
            # until the second phase and would stall the gate/up stream.
            gws = []
            for t in range(NHT):
                gw = wstream.tile([128, D // 128, 256], F8, tag="gw",
                                  name=f"gw{t}")
                nc.sync.dma_start(out=gw[:], in_=guw_d[t, :, :, :])
                gws.append(gw)
                if t == 0:
                    nc.sync.dma_start(out=tok8[:, 0:4, :],
                                      in_=tok8_d[:, 0:4, :])
                    nc.sync.dma_start(out=tok8[:, 4:8, :],
                                      in_=tok8_d[:, 4:8, :])
                    nc.sync.dma_start(out=wts[:], in_=wts_d[:, :])
                if t == 12:
                    # first half of the down weights: early enough that
                    # the down phase's first h-pairs aren't blocked, late
                    # enough not to starve the gate/up weight stream.
                    nc.sync.dma_start(out=dwn8[:, 0:4, :, :],
                                      in_=dwn8_d[:, 0:4, :, :])
            nc.sync.dma_start(out=dwn8[:, 4:8, :, :],
                              in_=dwn8_d[:, 4:8, :, :])

            pwB4 = None
            for t in range(NHT):
                gw = gws[t]
                # g/u psum: [0:512]=g, [512:1024]=u for the first 512
                # tokens (3-deep ring).  The 64-token tail chunks of four
                # consecutive h-tiles share one bank on the down ring and
                # get ONE batched silu+mul -- their results aren't needed
                # until the down phase, so this stays off the per-tile
                # critical chain.
                pwA = pgu.tile([128, 1024], F32, tag="guA")
                if t % 4 == 0:
                    pwB4 = po_p.tile([128, 512], F32, tag="o",
                                     name=f"pwB{t}")
                r = 64 * (t % 4)
                for gu in range(2):
                    csl = slice(gu * 128, gu * 128 + 128)
                    for c in range(NC2):
                        nc.tensor.matmul(
                            pwA[:, gu * 512:gu * 512 + 512],
                            gw[:, 2 * c:2 * c + 2, csl],
                            tok8[:, 2 * c:2 * c + 2, 0:512],
                            start=(c == 0), stop=(c == NC2 - 1),
                            perf_mode=PM.DoubleRow)
                    for c in range(NC2):
                        nc.tensor.matmul(
                            pwB4[:, 256 * gu + r:256 * gu + r + 64],
                            gw[:, 2 * c:2 * c + 2, csl],
                            tok8[:, 2 * c:2 * c + 2, 512:CAP],
                            start=(c == 0), stop=(c == NC2 - 1),
                            perf_mode=PM.DoubleRow)
                sg = sgp.tile([128, 512], BF16, tag="sg")
                nc.scalar.activation(out=sg[:], in_=pwA[:, 0:512],
                                     func=AF.Silu)
                nc.vector.tensor_mul(guT[:, t, 0:512], sg[:],
                                     pwA[:, 512:1024])
                if t % 4 == 3:
                    t0 = t - 3
                    sgB = sgp.tile([128, 256], BF16, tag="sgB")
                    nc.scalar.activation(out=sgB[:], in_=pwB4[:, 0:256],
                                         func=AF.Silu)
                    nc.vector.tensor_mul(
                        guT[:, t0:t0 + 4, 512:CAP],
                        sgB[:].rearrange("p (i d) -> p i d", d=64),
                        pwB4[:, 256:512].rearrange("p (i d) -> p i d",
                                                   d=64))

            for tt in range(NTT):
                ntok = min(128, CAP - tt * 128)
                tsl = slice(tt * 128, tt * 128 + ntok)
                ot = outp.tile([128, D], BF16, tag="ot")
                for half in range(2):
                    dsl = slice(half * 512, half * 512 + 512)
                    pso = po_p.tile([128, 512], F32, tag="o",
                                    name=f"pso{tt}_{half}")
                    for hp in range(H // 256):
                        nc.tensor.matmul(
                            pso[0:ntok, :], guT[:, 2 * hp:2 * hp + 2, tsl],
                            dwn8[:, hp, :, dsl],
                            start=(hp == 0), stop=(hp == H // 256 - 1),
                            perf_mode=PM.DoubleRow)
                    nc.vector.tensor_scalar_mul(ot[0:ntok, dsl],
                                                pso[0:ntok, :],
                                                wts[0:ntok, tt:tt + 1])
                    nc.sync.dma_start(out=eout_d[tsl, dsl],
                                      in_=ot[0:ntok, dsl])
    nc.compile()
    return nc


# --------------------------------------------------------------------------
# Host orchestration
# --------------------------------------------------------------------------

def _get(name, builder):
    if name not in _CACHE:
        _CACHE[name] = builder()
    return _CACHE[name]


def _attn_inputs(x2d, wq, bq, wkv, bkv, wo, norm1_w):
    """Build the 8 per-core input maps for launch A."""
    h = x2d.astype(np.float64)
    h = h / np.sqrt((h * h).mean(axis=-1, keepdims=True) + EPS)
    h = (h * norm1_w.astype(np.float64)).astype(np.float32)
    # hT[p, c, t] = h[t, 128c+p]
    hT = np.ascontiguousarray(
        h.T.reshape(D // 128, 128, T).transpose(1, 0, 2).astype(BF16_NP))

    wk = wkv[:, :D]
    wv = wkv[:, D:]
    bk = bkv[:D]

    tk = np.arange(128)[:, None]
    u = np.arange(128)[None, :]
    trimask = (u >= tk).astype(BF16_NP)
    onesb = np.ones((1, 128), BF16_NP)

    ins = []
    for c in range(NCORES):
        cs = slice(c * CW, (c + 1) * CW)
        packed = {n: np.ascontiguousarray(
            w[:, cs].reshape(D // 128, 128, CW).transpose(1, 0, 2)
            .astype(BF16_NP)) for n, w in (("wq", wq), ("wk", wk),
                                           ("wv", wv))}
        bqk_c = np.ascontiguousarray(
            np.stack([bq[cs], bk[cs]], axis=1).astype(np.float32))
        wo_c = np.ascontiguousarray(wo[cs, :].astype(BF16_NP))
        ins.append({
            "hT": hT,
            **packed,
            "bqk": bqk_c,
            "wo": wo_c,
            "trimask": trimask,
            "onesb": onesb,
        })
    return ins


def _route(x2, router_w, norm2_w):
    """Exact reference routing on host: rmsnorm2 + top-2 + softmax."""
    h2 = x2 / np.sqrt(np.mean(x2 * x2, axis=-1, keepdims=True) + EPS)
    h2 = (h2 * norm2_w).astype(np.float32)
    logits = h2.astype(np.float32) @ router_w.astype(np.float32)   # [N, E]
    idx1 = np.argmax(logits, axis=-1)
    l2 = logits.copy()
    l2[np.arange(T), idx1] = -np.inf
    idx2 = np.argmax(l2, axis=-1)
    v1 = logits[np.arange(T), idx1]
    v2 = logits[np.arange(T), idx2]
    # softmax over the two selected logits (v1 >= v2)
    e2 = np.exp((v2 - v1).astype(np.float32))
    p1 = (1.0 / (1.0 + e2)).astype(np.float32)
    p2 = (e2 / (1.0 + e2)).astype(np.float32)
    return h2, idx1, idx2, p1, p2


def kernel(x, freqs_cos, freqs_sin, norm1_w, wq, bq, wkv, bkv, wo, bo,
           norm2_w, router_w, gate_w, up_w, down_w):
    global MOE_ROUNDS
    x = np.asarray(x, np.float32)
    x2d = np.ascontiguousarray(x.reshape(T, D))
    wq = np.asarray(wq, np.float32)
    wkv = np.asarray(wkv, np.float32)
    wo = np.asarray(wo, np.float32)
    bq = np.asarray(bq, np.float32)
    bkv = np.asarray(bkv, np.float32)
    bo = np.asarray(bo, np.float32)
    norm1_w = np.asarray(norm1_w, np.float32)
    norm2_w = np.asarray(norm2_w, np.float32)
    router_w = np.asarray(router_w, np.float32)
    gate_w = np.asarray(gate_w, np.float32)
    up_w = np.asarray(up_w, np.float32)
    down_w = np.asarray(down_w, np.float32)

    # ---- launch A ----
    nc_a = _get("attn", build_attn)
    ins_a = _attn_inputs(x2d, wq, bq, wkv, bkv, wo, norm1_w)
    res_a = run_bass_kernel_spmd(nc_a, ins_a, core_ids=list(range(NCORES)))
    parts = np.stack([res_a.results[c]["part"].astype(np.float64)
                      for c in range(NCORES)])
    # v-bias folds through attention as +bv (softmax weights sum to 1),
    # so its wo image is added host-side along with bo.
    bv = bkv[D:].astype(np.float64)
    x2 = (x2d.astype(np.float64) + parts.sum(axis=0)
          + bv @ wo.astype(np.float64) + bo.astype(np.float64)
          ).astype(np.float32)

    # ---- host routing ----
    h2, idx1, idx2, p1, p2 = _route(x2, router_w, norm2_w)

    # per-expert token lists (order: top-1 hits then top-2 hits, stable)
    work = []   # (expert, token_idx array, weight array)
    for e in range(E):
        m1 = idx1 == e
        m2 = idx2 == e
        toks = np.concatenate([np.nonzero(m1)[0], np.nonzero(m2)[0]])
        wgts = np.concatenate([p1[m1], p2[m2]]).astype(np.float32)
        for s in range(0, max(len(toks), 1), CAP):
            work.append((e, toks[s:s + CAP], wgts[s:s + CAP]))

    h28 = h2.astype(F8_NP)
    guwb: dict = {}
    dwnb: dict = {}
    NTT = (CAP + 127) // 128

    # ---- launch B (one round of 8 unless an expert overflows CAP) ----
    nc_b = _get("moe", build_moe)
    moe = np.zeros((T, D), np.float64)
    MOE_ROUNDS = 0
    for r0 in range(0, len(work), NCORES):
        batch = work[r0:r0 + NCORES]
        while len(batch) < NCORES:
            batch.append((0, np.zeros(0, np.int64), np.zeros(0, np.float32)))
        ins_b = []
        for e, toks, wgts in batch:
            tok8 = np.zeros((128, D // 128, CAP), F8_NP)
            tok8t = h28[toks].T.reshape(D // 128, 128, len(toks))
            tok8[:, :, :len(toks)] = tok8t.transpose(1, 0, 2)
            wts = np.zeros((NTT * 128,), np.float32)
            wts[:len(toks)] = wgts * MOE_SCALE
            if e not in guwb:
                gu = np.concatenate([
                    gate_w[e].reshape(D, H // 128, 128),
                    up_w[e].reshape(D, H // 128, 128)], axis=2)  # [D,16,256]
                guwb[e] = np.ascontiguousarray(
                    gu.reshape(D // 128, 128, H // 128, 256)
                    .transpose(2, 1, 0, 3).astype(F8_NP))
                dwnb[e] = np.ascontiguousarray(
                    down_w[e].reshape(H // 256, 2, 128, D)
                    .transpose(2, 0, 1, 3).astype(F8_NP))
            ins_b.append({
                "tok8": tok8,
                "guw": guwb[e],
                "dwn8": dwnb[e],
                "wts": np.ascontiguousarray(
                    wts.reshape(NTT, 128).T.astype(np.float32)),
            })
        res_b = run_bass_kernel_spmd(nc_b, ins_b, core_ids=list(range(NCORES)))
        MOE_ROUNDS += 1
        for (e, toks, wgts), rc in zip(batch, res_b.results):
            if len(toks):
                moe[toks] += rc["eout"][:len(toks)].astype(np.float64)

    out = (x2.astype(np.float64) + moe).astype(np.float32)
    return out.reshape(B, T, D)
